# revision 18
# baseline (speedup 1.0000x reference)
"""Trainium2 Bass kernel: causal multi-head attention block (B=2, T=2048, C=1024, H=16).

Sharding: 8 cores = 2 (batch) x 4 (head groups of 4 heads).  Each core computes
q/k/v projections for its 4 heads, causal attention, and a partial out-proj
(rows of wo for its head slice).  Host sums the 4 partials per batch element.

v6: v3's software-pipelined schedule plus two PE packs, with precision kept
where fp8 noise does not average out:
  - q/k projections in fp8e4 with perf_mode=DoubleRow (contraction pairs
    packed 2-per-PE-cell: 4 matmuls per 1024-deep projection instead of 8).
    Weights scaled x64 on the host (fp8e4 subnormal cutoff 2^-6); the 1/64
    rides the existing bias-add.  Softmax is insensitive to the ~4% fp8
    element noise on scores (row-common factors cancel; diffuse rows
    average), unlike the v/out paths where fp8 noise lands directly on the
    output (measured ~5e-2 rel err) — so v-proj and out-proj stay bf16.
  - scores via PE row tiling: head A in PE rows 0:64, head B in rows 64:128
    (qT/kT stored as stacked pair tiles); the two score matmuls issue
    back-to-back and run concurrently in the array (~2x).  Head B keeps its
    own PSUM bank (same-bank packing + concurrency hangs the NEFF).
  - one strided exp activation per j-step ([A|B] banks in a single 3D AP);
    Act engine runs exp only, copies live on DVE.

Per-core layouts:
  x8      [128, 4, 2, 2048]  x[b].T partition-major c-pairs      (fp8e4)
  w8      [128, 4, 2, 512]   [64*wq_s.T | 64*wk_s.T] pairs       (fp8e4)
  xT      [1024, 2048]       x[b].T                              (bf16)
  wvT     [1024, 256]        wv_s.T                              (bf16)
  woT     [256, 1024]        wo[:, head_slice].T                 (bf16)
  bqk     [128, 4]  cols: bq/8 (pair0,pair1), bk (pair0,pair1)   (f32)
  bv_row  [1, 256]  bv                                           (f32)
  y       [2048, 1024]  partial output (pre-sum, pre-bo)         (bf16)
"""

import os
import sys

import numpy as np
import ml_dtypes

F8_NP = ml_dtypes.float8_e4m3   # TRN fp8e4: max 240, inf at 256
BF16_NP = ml_dtypes.bfloat16

for _p in ("/opt/trn_rl_repo", "/root/.axon_site/_ro/trn_rl_repo"):
    if os.path.isdir(_p) and _p not in sys.path:
        sys.path.append(_p)

import concourse.bass as bass  # noqa: E402
import concourse.mybir as mybir  # noqa: E402
import concourse.tile as tile  # noqa: E402

F32 = mybir.dt.float32
BF16 = mybir.dt.bfloat16
FP8 = mybir.dt.float8e4
DR = mybir.MatmulPerfMode.DoubleRow
MUL = mybir.AluOpType.mult
ADD = mybir.AluOpType.add

B, T, C, H = 2, 2048, 1024, 16
D = C // H          # 64
HPC = 4             # heads per core
DPC = HPC * D       # 256 head-dims per core
NCORES = 8

CHUNK = 128         # s-chunk / contraction granularity
SST = 512           # attention t-strip == one PSUM bank of f32
VW = 2 * D          # 128 per head: [ones x64 | v x64]
WSC = 64.0          # host-side fp8 weight scale

_CTRL_TYPES = (mybir.InstDrain, mybir.InstNoOp, mybir.InstEventSemaphore)


def split_excess_waits(nc, lim=1):
    """Walrus accepts at most one sync-wait per instruction; move extras onto
    same-engine NoOps inserted just before the owner."""
    k = 0
    for fn in nc.m.functions:
        for blk in fn.blocks:
            out = []
            changed = False
            for inst in blk.instructions:
                si = inst.sync_info
                if si is not None and si.on_wait and len(si.on_wait) > lim:
                    waits = list(si.on_wait)
                    extra, keep = waits[:-lim], waits[-lim:]
                    for w in extra:
                        nop = mybir.InstNoOp(name=f"waitfix_{k}", ins=[], outs=[])
                        k += 1
                        nop.engine = inst.engine
                        nop.sync_info = mybir.SyncInfo(on_wait=[w], on_update=[])
                        out.append(nop)
                    si.on_wait = keep
                    changed = True
                out.append(inst)
            if changed:
                blk.instructions = out
    return k


def build_nc(t_len=T, fix_waits=True):
    """Build the per-core SPMD Bass program (same program on all 8 cores)."""
    assert t_len % SST == 0
    nks = t_len // SST                # 4 strips
    n_cchunk = C // CHUNK             # 8
    n_ttile = t_len // CHUNK          # 16
    NCP = C // 256                    # 4 contraction pairs

    nc = bass.Bass(target_bir_lowering=False)

    x8 = nc.dram_tensor("x8", [CHUNK, NCP, 2, t_len], FP8, kind="ExternalInput")
    w8 = nc.dram_tensor("w8", [CHUNK, NCP, 2, 2 * DPC], FP8, kind="ExternalInput")
    xT = nc.dram_tensor("xT", [C, t_len], BF16, kind="ExternalInput")
    wvT = nc.dram_tensor("wvT", [C, DPC], BF16, kind="ExternalInput")
    woT = nc.dram_tensor("woT", [DPC, C], BF16, kind="ExternalInput")
    bqk = nc.dram_tensor("bqk", [CHUNK, 4], F32, kind="ExternalInput")
    bv_row = nc.dram_tensor("bv_row", [1, DPC], F32, kind="ExternalInput")
    y = nc.dram_tensor("y", [t_len, C], BF16, kind="ExternalOutput")

    Exp = mybir.ActivationFunctionType.Exp
    XW = 2 * NCP * SST               # per-strip fp8 x tile cols
    WW = 2 * NCP * 2 * DPC

    with tile.TileContext(nc) as tc:
        with tc.tile_pool(name="persist", bufs=1) as pp, \
             tc.tile_pool(name="work", bufs=1) as wp, \
             tc.tile_pool(name="dr", bufs=1, space="DRAM") as dr, \
             tc.tile_pool(name="ps", bufs=1, space="PSUM") as ps:
            # scalar-engine warmup: trigger the exp table load early
            warm = pp.tile([1, 8], F32, tag="warm", name="warm")
            nc.gpsimd.memset(warm, 1.0)
            nc.scalar.activation(warm, warm, mybir.ActivationFunctionType.Ln)
            nc.scalar.activation(warm, warm, Exp)

            # ---- input DMAs (spread across idle engine queues) ----
            bqk_sb = pp.tile([CHUNK, 4], F32, tag="bqk", name="bqk_sb")
            bv_bc = pp.tile([CHUNK, DPC], F32, tag="bv_bc", name="bv_bc")

            def load_x8(strip, parts, engs):
                """Load fp8 x strip (all NCP c-pairs) into one wide tile."""
                xts = wp.tile([CHUNK, XW], FP8, tag="x8s", bufs=2,
                              name=f"x8s_{strip}")
                c0 = 0
                for i, n in enumerate(parts):
                    engs[i % len(engs)].dma_start(
                        out=xts[:, c0 * 2 * SST:(c0 + n) * 2 * SST].rearrange(
                            "p (c i t) -> p c i t", i=2, t=SST),
                        in_=x8[:, c0:c0 + n, :,
                               strip * SST:(strip + 1) * SST])
                    c0 += n
                return xts

            def load_xb(strip, parts, engs):
                """Load bf16 x strip row-chunks (for the v projection)."""
                xbs = wp.tile([CHUNK, n_cchunk * SST], BF16, tag="xbs", bufs=2,
                              name=f"xbs_{strip}")
                c0 = 0
                for i, n in enumerate(parts):
                    engs[i % len(engs)].dma_start(
                        out=xbs[:, c0 * SST:(c0 + n) * SST].rearrange(
                            "p (c t) -> p c t", t=SST),
                        in_=xT[c0 * CHUNK:(c0 + n) * CHUNK,
                               strip * SST:(strip + 1) * SST].rearrange(
                            "(c p) t -> p c t", p=CHUNK))
                    c0 += n
                return xbs

            nc.gpsimd.dma_start(out=bqk_sb, in_=bqk[:, :])
            nc.gpsimd.dma_start(out=bv_bc, in_=bv_row[0:1, :].broadcast_to((CHUNK, DPC)))
            wts = pp.tile([CHUNK, WW], FP8, tag="wts", name="wts")
            wvb = pp.tile([CHUNK, n_cchunk * DPC], BF16, tag="wvb", name="wvb")
            x8s0 = wp.tile([CHUNK, XW], FP8, tag="x8s", bufs=2, name="x8s_0")
            rr = [nc.gpsimd, nc.sync, nc.scalar]
            qi = 0
            for c in range(NCP):
                rr[qi % 3].dma_start(
                    out=wts[:, c * 2 * 2 * DPC:(c + 1) * 2 * 2 * DPC],
                    in_=w8[:, c, :, :])
                qi += 1
                rr[qi % 3].dma_start(
                    out=x8s0[:, c * 2 * SST:(c + 1) * 2 * SST].rearrange(
                        "p (i t) -> p i t", i=2),
                    in_=x8[:, c, :, 0:SST])
                qi += 1
            nc.gpsimd.dma_start(
                out=wvb.rearrange("p (c w) -> p c w", w=DPC),
                in_=wvT[:, :].rearrange("(c p) w -> p c w", p=CHUNK))
            xb0 = load_xb(0, [4, 4], [nc.sync, nc.scalar])
            wtv = wts.rearrange("p (c i w) -> p c i w", i=2, w=2 * DPC)
            wv_sb = [wvb[:, c * DPC:(c + 1) * DPC] for c in range(n_cchunk)]

            def xtv(xts):
                return xts.rearrange("p (c i t) -> p c i t", i=2, t=SST)

            x8_bufs = {0: x8s0}
            xb_bufs = {0: xb0}
            if nks > 1:
                x8_bufs[1] = load_x8(1, [2, 2], [nc.sync, nc.scalar])
                xb_bufs[1] = load_xb(1, [4, 4], [nc.sync, nc.scalar])

            wot = pp.tile([CHUNK, 2 * C], BF16, tag="wot", name="wot")
            wo_sb = [wot[:, 0:C], wot[:, C:2 * C]]

            def load_wo():
                nc.gpsimd.dma_start(
                    out=wot.rearrange("p (c w) -> p c w", w=C),
                    in_=woT[:, :].rearrange("(c p) w -> p c w", p=CHUNK))

            # HAM warm-up: keep the PE busy on throwaway matmuls while the
            # input DMAs land, so real work starts at 2.4 GHz
            dum = pp.tile([CHUNK, SST], BF16, tag="dum", name="dum")
            nc.vector.memset(dum, 0.0)
            for i in range(10):
                pd = ps.tile([CHUNK, SST], F32, tag="fill", bufs=2,
                             name=f"pd_{i}")
                nc.tensor.matmul(pd, dum[:, 0:CHUNK], dum, start=True, stop=True)

            # ---- persistent activations ----
            # qT2/kT2[m]: heads 2m / 2m+1 stacked on partitions 0:64 / 64:128
            qT2 = [pp.tile([CHUNK, t_len], BF16, tag=f"qT{m}", name=f"qT{m}")
                   for m in range(2)]
            kT2 = [pp.tile([CHUNK, t_len], BF16, tag=f"kT{m}", name=f"kT{m}")
                   for m in range(2)]
            vaug = [pp.tile([CHUNK, HPC * VW], BF16, tag=f"v{j}", name=f"v{j}")
                    for j in range(n_ttile)]
            for j in range(n_ttile):
                eng = nc.vector if j % 2 == 0 else nc.gpsimd
                eng.memset(vaug[j], 1.0)   # ones half survives; rest overwritten
            aoT = [pp.tile([CHUNK, t_len], BF16, tag=f"aoT{p}", name=f"aoT{p}")
                   for p in range(2)]
            # 0/1 lower-triangle for diagonal-tile masking (DVE multiply —
            # keeps the gpsimd queue out of the per-step critical path)
            tri = pp.tile([CHUNK, CHUNK], BF16, tag="tri", name="tri")
            nc.gpsimd.memset(tri, 1.0)
            nc.gpsimd.affine_select(
                out=tri, in_=tri, pattern=[[1, CHUNK]], channel_multiplier=-1,
                base=0, compare_op=mybir.AluOpType.is_ge, fill=0.0)

            # =========== emit helpers ===========
            def proj_qk_group(strip, pj, m, xts):
                """q or k (pj=0/1) for head-pair m over one t-strip: fp8 DR."""
                xt = xtv(xts)
                pq = ps.tile([CHUNK, SST], F32, tag="fill", bufs=2,
                             name=f"pq{pj}{m}_{strip}")
                base = pj * DPC + m * CHUNK
                for cp in range(NCP):
                    nc.tensor.matmul(
                        pq,
                        wtv[:, cp, :, base:base + CHUNK],
                        xt[:, cp, :, :],
                        start=(cp == 0), stop=(cp == NCP - 1),
                        perf_mode=DR)
                dst = kT2[m] if pj else qT2[m]
                # q additionally carries the softmax 1/sqrt(64)=1/8
                sc = 1.0 / (WSC * 8.0) if pj == 0 else 1.0 / WSC
                nc.vector.tensor_scalar(
                    dst[:, strip * SST:(strip + 1) * SST], pq,
                    sc, bqk_sb[:, 2 * pj + m:2 * pj + m + 1], MUL, ADD)

            def proj_v_group(strip, u, xbs):
                """v for t-chunk 4*strip+u -> vaug (bf16 matmuls)."""
                jt = 4 * strip + u
                pv = ps.tile([CHUNK, SST], F32, tag="fill", bufs=2,
                             name=f"pv_{jt}")
                for c in range(n_cchunk):
                    nc.tensor.matmul(
                        pv[:, 0:DPC],
                        xbs[:, c * SST + u * CHUNK:c * SST + (u + 1) * CHUNK],
                        wv_sb[c],
                        start=(c == 0), stop=(c == n_cchunk - 1))
                nc.vector.tensor_add(
                    vaug[jt].rearrange("p (h e) -> p h e", e=VW)[:, :, D:2 * D],
                    pv[:, 0:DPC].rearrange("p (h d) -> p h d", d=D),
                    bv_bc.rearrange("p (h d) -> p h d", d=D))

            def outproj_group(jt, tail=False):
                for js in range(2):
                    py = ps.tile([CHUNK, SST], F32, tag="fill", bufs=2,
                                 name=f"py_{jt}_{js}")
                    for p in range(2):
                        nc.tensor.matmul(
                            py,
                            aoT[p][:, jt * CHUNK:(jt + 1) * CHUNK],
                            wo_sb[p][:, js * SST:(js + 1) * SST],
                            start=(p == 0), stop=(p == 1))
                    ysb = wp.tile([CHUNK, SST], BF16, tag="ysb", bufs=3,
                                  name=f"ysb_{jt}_{js}")
                    if tail and (2 * jt + js) % 2 == 0:
                        # the scalar engine is idle in the tail; splitting the
                        # PSUM evictions halves the vector-bound epilogue
                        nc.scalar.activation(ysb, py,
                                             mybir.ActivationFunctionType.Copy)
                    else:
                        nc.vector.tensor_copy(ysb, py)
                    if tail:
                        eng = [nc.sync, nc.scalar][(2 * jt + js) % 2]
                    else:
                        eng = [nc.gpsimd, nc.sync][(2 * jt + js) % 2]
                    eng.dma_start(
                        out=y[jt * CHUNK:(jt + 1) * CHUNK,
                              js * SST:(js + 1) * SST],
                        in_=ysb)

            pend = {}

            def scores_step(p, ks, j):
                """scores -> exp -> (mask) for one s-chunk j of strip ks.

                Scores for heads 2p / 2p+1 run concurrently via PE row tiling
                (kT2/qT2 partition halves); head B in its own PSUM bank.
                One strided exp covers both banks; eAB packs [A | B] at L.
                The AV consuming eAB is emitted one j-step later (av_step) so
                the in-order tensor queue never waits on this step's exp.
                """
                off = max(0, CHUNK * j - SST * ks)
                L = SST - off
                t0 = SST * ks + off
                jc = j * CHUNK
                sAB = ps.tile([CHUNK, 2 * SST], F32, tag="sAB", bufs=2,
                              name=f"s_{p}_{ks}_{j}")
                nc.tensor.matmul(
                    sAB[:, 0:L],
                    kT2[p][0:D, jc:jc + CHUNK],
                    qT2[p][0:D, t0:t0 + L],
                    start=True, stop=True, skip_group_check=True)
                nc.tensor.matmul(
                    sAB[:, SST:SST + L],
                    kT2[p][D:CHUNK, jc:jc + CHUNK],
                    qT2[p][D:CHUNK, t0:t0 + L],
                    start=True, stop=True, skip_group_check=True)
                eAB = wp.tile([CHUNK, 2 * SST], BF16, tag="eAB", bufs=3,
                              name=f"e_{p}_{ks}_{j}")
                if L == SST:
                    nc.scalar.activation(eAB, sAB, Exp)
                else:
                    nc.scalar.activation(
                        eAB[:, 0:2 * L].rearrange("p (c t) -> p c t", c=2),
                        sAB.rearrange("p (c t) -> p c t", t=SST)[:, :, 0:L],
                        Exp)
                if CHUNK * j >= SST * ks:  # diagonal tile: zero upper triangle
                    for base in (0, L):
                        nc.vector.tensor_mul(
                            eAB[:, base:base + CHUNK],
                            eAB[:, base:base + CHUNK], tri)
                pend[j] = (eAB, off, L)

            def av_step(p, ks, j, nj):
                hA, hB = 2 * p, 2 * p + 1
                eAB, off, L = pend.pop(j)
                av = av_cur[0]
                nc.tensor.matmul(
                    av[:, off:SST],
                    vaug[j][:, hA * VW:(hA + 1) * VW],
                    eAB[:, 0:L],
                    start=(j == 0), stop=(j == nj - 1), skip_group_check=True)
                nc.tensor.matmul(
                    av[:, SST + off:2 * SST],
                    vaug[j][:, hB * VW:(hB + 1) * VW],
                    eAB[:, L:2 * L],
                    start=(j == 0), stop=(j == nj - 1), skip_group_check=True)

            pending_mul = []

            def flush_mul():
                while pending_mul:
                    p, ks, bcf = pending_mul.pop(0)
                    strip = slice(ks * SST, (ks + 1) * SST)
                    half = SST // 2
                    lo = ks * SST
                    nc.gpsimd.tensor_mul(aoT[p][:, lo:lo + half],
                                         aoT[p][:, lo:lo + half],
                                         bcf[:, 0:half])
                    nc.gpsimd.tensor_mul(aoT[p][:, lo + half:lo + SST],
                                         aoT[p][:, lo + half:lo + SST],
                                         bcf[:, half:SST])

            def normalize(p, ks, tail=False):
                """softmax-normalize strip ks of pair p into aoT[p].

                av rows 0:64 hold the PE-replicated denominators, rows 64:128
                the unnormalized outputs.  Reciprocals run directly on the
                PSUM denominator rows (the DVE 32-lane shuffle crossbar
                handles the 64-partition shift), no DRAM bounce needed.  The
                final aoT multiply is deferred (flush_mul) so it never blocks
                the next pair's masks on the gpsimd queue — except in the
                tail, where it runs immediately."""
                av = av_cur[0]
                strip = slice(ks * SST, (ks + 1) * SST)
                half = SST // 2
                lo = ks * SST
                if tail:
                    # low-latency path: 1/d = exp(-log d) on the (idle) scalar
                    # (Ln of the A-half reads the PSUM denominator directly;
                    # the B-half needs the DVE crossbar for the 64-partition
                    # shift, so it goes through a copy)
                    rec = wp.tile([CHUNK, SST], F32, tag="rec", bufs=2,
                                  name=f"rec_{p}_{ks}")
                    nc.scalar.activation(rec[0:D, :], av[0:D, 0:SST],
                                         mybir.ActivationFunctionType.Ln)
                    nc.vector.tensor_copy(rec[D:CHUNK, :], av[0:D, SST:2 * SST])
                    nc.vector.tensor_copy(aoT[p][0:D, strip],
                                          av[D:CHUNK, 0:SST])
                    nc.scalar.activation(rec[D:CHUNK, :], rec[D:CHUNK, :],
                                         mybir.ActivationFunctionType.Ln)
                    nc.vector.tensor_copy(aoT[p][D:CHUNK, strip],
                                          av[D:CHUNK, SST:2 * SST])
                    nc.scalar.activation(rec, rec, Exp, scale=-1.0)
                    nc.gpsimd.tensor_mul(aoT[p][:, lo:lo + half],
                                         aoT[p][:, lo:lo + half],
                                         rec[:, 0:half])
                    nc.vector.tensor_mul(aoT[p][:, lo + half:lo + SST],
                                         aoT[p][:, lo + half:lo + SST],
                                         rec[:, half:SST])
                    return
                den = wp.tile([1, 2 * SST], F32, tag="den", bufs=2,
                              name=f"den_{p}_{ks}")
                # evicts; frees the av banks.  B-side evict on the scalar
                # engine (idle at pair boundaries).
                nc.vector.tensor_copy(aoT[p][0:D, strip],
                                      av[D:CHUNK, 0:SST])
                nc.scalar.activation(aoT[p][D:CHUNK, strip],
                                     av[D:CHUNK, SST:2 * SST],
                                     mybir.ActivationFunctionType.Copy)
                nc.vector.tensor_copy(den[0:1, 0:SST], av[0:1, 0:SST])
                nc.vector.tensor_copy(den[0:1, SST:2 * SST],
                                      av[0:1, SST:2 * SST])
                # reciprocal via a [128, 8] reshape (wide on the DVE lanes),
                # then partition-broadcast — both through small DRAM bounces
                # (SBUF APs cannot reshape across partitions / stride-0 bcast).
                # Latency is hidden: the aoT multiply is deferred into the
                # next pair's j-loop (flush_mul).
                dden = dr.tile([1, 2 * SST], F32, tag="dden", bufs=2,
                               name=f"dden_{p}_{ks}")
                nc.sync.dma_start(out=dden, in_=den)
                nf = 2 * SST // CHUNK
                dsb = wp.tile([CHUNK, nf], F32, tag="dsb", bufs=2,
                              name=f"dsb_{p}_{ks}")
                dview = dden.rearrange("a b -> (a b)").rearrange(
                    "(p f) -> p f", p=CHUNK)
                nc.sync.dma_start(out=dsb, in_=dview)
                rsb = wp.tile([CHUNK, nf], F32, tag="rsb", bufs=2,
                              name=f"rsb_{p}_{ks}")
                nc.vector.reciprocal(rsb, dsb)
                drec = dr.tile([1, 2 * SST], F32, tag="drec", bufs=2,
                               name=f"drec_{p}_{ks}")
                rview = drec.rearrange("a b -> (a b)").rearrange(
                    "(p f) -> p f", p=CHUNK)
                nc.sync.dma_start(out=rview, in_=rsb)
                bcf = wp.tile([CHUNK, SST], F32, tag="bcf", bufs=2,
                              name=f"bcf_{p}_{ks}")
                nc.sync.dma_start(
                    out=bcf[0:D, :],
                    in_=drec[0:1, 0:SST].broadcast_to((D, SST)))
                nc.sync.dma_start(
                    out=bcf[D:CHUNK, :],
                    in_=drec[0:1, SST:2 * SST].broadcast_to((D, SST)))
                pending_mul.append((p, ks, bcf))

            # =========== schedule ===========
            # pre-loop: full projection of strip 0
            for pj in (1, 0):
                for m in range(2):
                    proj_qk_group(0, pj, m, x8_bufs[0])
            for u in range(4):
                proj_v_group(0, u, xb_bufs[0])

            av_cur = [None]
            for ks in range(nks):
                if ks == 0:
                    load_wo()
                # prefetch x strip ks+2 (its buffers were freed by proj(ks))
                if ks + 2 < nks:
                    x8_bufs[ks + 2] = load_x8(ks + 2, [2, 2], [nc.scalar])
                    xb_bufs[ks + 2] = load_xb(ks + 2, [4, 4], [nc.scalar])

                fillers = []
                if ks + 1 < nks:                  # projection of next strip
                    for pj in (1, 0):
                        for m in range(2):
                            fillers.append(
                                (proj_qk_group, (ks + 1, pj, m, x8_bufs[ks + 1])))
                    for u in range(4):
                        fillers.append(
                            (proj_v_group, (ks + 1, u, xb_bufs[ks + 1])))
                if ks == nks - 1:                 # deferred out-proj
                    for jt in range(4 * (nks - 1)):
                        fillers.append((outproj_group, (jt,)))

                nj = 4 * ks + 4
                total_js = 2 * nj
                gi = 0
                cnt = 0
                for p in (0, 1):
                    av_cur[0] = ps.tile([CHUNK, 2 * SST], F32, tag="av", bufs=1,
                                        name=f"av_{p}_{ks}")
                    res = (4 if p == 0 else 2) if ks == nks - 1 else 2
                    cap = max(0, len(fillers) - res)
                    for j in range(nj):
                        scores_step(p, ks, j)
                        if j > 1:
                            av_step(p, ks, j - 2, nj)
                        if j == 3:
                            flush_mul()   # previous pair's deferred aoT mul
                        cnt += 1
                        want = min(cap,
                                   ((cnt + 2) * len(fillers)) // total_js)
                        while gi < want:
                            fn, args = fillers[gi]
                            fn(*args)
                            gi += 1
                    av_step(p, ks, nj - 2, nj)
                    av_step(p, ks, nj - 1, nj)
                    normalize(p, ks, tail=(ks == nks - 1 and p == 1))
                    # keep the tensor queue fed while av drains
                    for _ in range(res):
                        if gi < len(fillers):
                            fn, args = fillers[gi]
                            fn(*args)
                            gi += 1
                while gi < len(fillers):
                    fn, args = fillers[gi]
                    fn(*args)
                    gi += 1

            # tail: out-proj of the last strip
            for jt in range(4 * (nks - 1), 4 * nks):
                outproj_group(jt, tail=True)

    if fix_waits:
        split_excess_waits(nc)
    return nc


def make_in_maps(x, wq, bq, wk, bk, wv, bv, wo, bo, t_len=T):
    """Build the 8 per-core input dicts from full inputs."""
    in_maps = []
    NCP = C // 256

    for core in range(NCORES):
        b, hg = core // 4, core % 4
        sl = slice(DPC * hg, DPC * (hg + 1))
        wqk = np.concatenate(
            [wq[sl].T, wk[sl].T], axis=1).astype(np.float32) * WSC
        w8 = np.ascontiguousarray(
            wqk.reshape(NCP, 2, CHUNK, -1).transpose(2, 0, 1, 3)).astype(F8_NP)
        bqs = (bq[sl] / 8.0).astype(np.float32)
        bqkm = np.stack([bqs[0:CHUNK], bqs[CHUNK:2 * CHUNK],
                         bk[sl][0:CHUNK], bk[sl][CHUNK:2 * CHUNK]], axis=1)
        xTb = np.ascontiguousarray(x[b, :t_len].T)
        x8r = np.ascontiguousarray(
            xTb.reshape(NCP, 2, CHUNK, t_len).transpose(2, 0, 1, 3)
        ).astype(F8_NP)
        in_maps.append({
            "x8": x8r,
            "w8": w8,
            "xT": xTb.astype(BF16_NP),
            "wvT": np.ascontiguousarray(wv[sl].T).astype(BF16_NP),
            "woT": np.ascontiguousarray(wo[:, sl].T).astype(BF16_NP),
            "bqk": np.ascontiguousarray(bqkm, dtype=np.float32),
            "bv_row": np.ascontiguousarray(bv[sl][None, :], dtype=np.float32),
        })
    return in_maps


def gather_output(results, bo, t_len=T):
    ys = [np.asarray(results[i]["y"], dtype=np.float32) for i in range(NCORES)]
    out = np.stack([ys[0] + ys[1] + ys[2] + ys[3],
                    ys[4] + ys[5] + ys[6] + ys[7]])
    out += np.asarray(bo, np.float32)[None, None, :]
    return out


_NC_CACHE = {}


def _get_nc(t_len=T):
    if t_len not in _NC_CACHE:
        _NC_CACHE[t_len] = build_nc(t_len)
    return _NC_CACHE[t_len]


def kernel(x, wq, bq, wk, bk, wv, bv, wo, bo, mask=None, **_unused):
    """Full-input entry point: shard, run on 8 NeuronCores, gather."""
    from concourse.bass_utils import run_bass_kernel_spmd

    x = np.asarray(x, dtype=np.float32)
    in_maps = make_in_maps(x, np.asarray(wq, np.float32), np.asarray(bq, np.float32),
                           np.asarray(wk, np.float32), np.asarray(bk, np.float32),
                           np.asarray(wv, np.float32), np.asarray(bv, np.float32),
                           np.asarray(wo, np.float32), np.asarray(bo, np.float32))
    nc = _get_nc(T)
    res = run_bass_kernel_spmd(nc, in_maps, list(range(NCORES)))
    return gather_output(res.results, bo)


# revision 26
# speedup vs baseline: 1.0407x; 1.0407x over previous
"""Trainium2 Bass kernel: causal multi-head attention block (B=2, T=2048, C=1024, H=16).

Sharding: 8 cores = 2 (batch) x 4 (head groups of 4 heads).  Each core computes
q/k/v projections for its 4 heads, causal attention, and a partial out-proj
(rows of wo for its head slice).  Host sums the 4 partials per batch element.

v6: v3's software-pipelined schedule plus two PE packs, with precision kept
where fp8 noise does not average out:
  - q/k projections in fp8e4 with perf_mode=DoubleRow (contraction pairs
    packed 2-per-PE-cell: 4 matmuls per 1024-deep projection instead of 8).
    Weights scaled x64 on the host (fp8e4 subnormal cutoff 2^-6); the 1/64
    rides the existing bias-add.  Softmax is insensitive to the ~4% fp8
    element noise on scores (row-common factors cancel; diffuse rows
    average), unlike the v/out paths where fp8 noise lands directly on the
    output (measured ~5e-2 rel err) — so v-proj and out-proj stay bf16.
  - scores via PE row tiling: head A in PE rows 0:64, head B in rows 64:128
    (qT/kT stored as stacked pair tiles); the two score matmuls issue
    back-to-back and run concurrently in the array (~2x).  Head B keeps its
    own PSUM bank (same-bank packing + concurrency hangs the NEFF).
  - one strided exp activation per j-step ([A|B] banks in a single 3D AP);
    Act engine runs exp only, copies live on DVE.

Per-core layouts:
  x8      [128, 4, 2, 2048]  x[b].T partition-major c-pairs      (fp8e4)
  w8      [128, 4, 2, 512]   [64*wq_s.T | 64*wk_s.T] pairs       (fp8e4)
  xT      [1024, 2048]       x[b].T                              (bf16)
  wvT     [1024, 256]        wv_s.T                              (bf16)
  woT     [256, 1024]        wo[:, head_slice].T                 (bf16)
  bqk     [128, 4]  cols: bq/8 (pair0,pair1), bk (pair0,pair1)   (f32)
  bv_row  [1, 256]  bv                                           (f32)
  y       [2048, 1024]  partial output (pre-sum, pre-bo)         (bf16)
"""

import os
import sys

import numpy as np
import ml_dtypes

F8_NP = ml_dtypes.float8_e4m3   # TRN fp8e4: max 240, inf at 256
BF16_NP = ml_dtypes.bfloat16

for _p in ("/opt/trn_rl_repo", "/root/.axon_site/_ro/trn_rl_repo"):
    if os.path.isdir(_p) and _p not in sys.path:
        sys.path.append(_p)

import concourse.bass as bass  # noqa: E402
import concourse.mybir as mybir  # noqa: E402
import concourse.tile as tile  # noqa: E402

F32 = mybir.dt.float32
BF16 = mybir.dt.bfloat16
FP8 = mybir.dt.float8e4
DR = mybir.MatmulPerfMode.DoubleRow
MUL = mybir.AluOpType.mult
ADD = mybir.AluOpType.add

B, T, C, H = 2, 2048, 1024, 16
D = C // H          # 64
HPC = 4             # heads per core
DPC = HPC * D       # 256 head-dims per core
NCORES = 8

CHUNK = 128         # s-chunk / contraction granularity
SST = 512           # attention t-strip == one PSUM bank of f32
VW = 2 * D          # 128 per head: [ones x64 | v x64]
WSC = 64.0          # host-side fp8 weight scale

_CTRL_TYPES = (mybir.InstDrain, mybir.InstNoOp, mybir.InstEventSemaphore)


def split_excess_waits(nc, lim=1):
    """Walrus accepts at most one sync-wait per instruction; move extras onto
    same-engine NoOps inserted just before the owner."""
    k = 0
    for fn in nc.m.functions:
        for blk in fn.blocks:
            out = []
            changed = False
            for inst in blk.instructions:
                si = inst.sync_info
                if si is not None and si.on_wait and len(si.on_wait) > lim:
                    waits = list(si.on_wait)
                    extra, keep = waits[:-lim], waits[-lim:]
                    for w in extra:
                        nop = mybir.InstNoOp(name=f"waitfix_{k}", ins=[], outs=[])
                        k += 1
                        nop.engine = inst.engine
                        nop.sync_info = mybir.SyncInfo(on_wait=[w], on_update=[])
                        out.append(nop)
                    si.on_wait = keep
                    changed = True
                out.append(inst)
            if changed:
                blk.instructions = out
    return k


def build_nc(t_len=T, fix_waits=True):
    """Build the per-core SPMD Bass program (same program on all 8 cores)."""
    assert t_len % SST == 0
    nks = t_len // SST                # 4 strips
    n_cchunk = C // CHUNK             # 8
    n_ttile = t_len // CHUNK          # 16
    NCP = C // 256                    # 4 contraction pairs

    nc = bass.Bass(target_bir_lowering=False)

    x8 = nc.dram_tensor("x8", [CHUNK, NCP, 2, t_len], FP8, kind="ExternalInput")
    w8 = nc.dram_tensor("w8", [CHUNK, NCP, 2, 2 * DPC], FP8, kind="ExternalInput")
    xT = nc.dram_tensor("xT", [C, t_len], BF16, kind="ExternalInput")
    wvT = nc.dram_tensor("wvT", [C, DPC], BF16, kind="ExternalInput")
    woT = nc.dram_tensor("woT", [DPC, C], BF16, kind="ExternalInput")
    bqk = nc.dram_tensor("bqk", [CHUNK, 4], F32, kind="ExternalInput")
    bv_row = nc.dram_tensor("bv_row", [1, DPC], F32, kind="ExternalInput")
    y = nc.dram_tensor("y", [t_len, C], BF16, kind="ExternalOutput")

    Exp = mybir.ActivationFunctionType.Exp
    XW = 2 * NCP * SST               # per-strip fp8 x tile cols
    WW = 2 * NCP * 2 * DPC

    with tile.TileContext(nc) as tc:
        with tc.tile_pool(name="persist", bufs=1) as pp, \
             tc.tile_pool(name="work", bufs=1) as wp, \
             tc.tile_pool(name="dr", bufs=1, space="DRAM") as dr, \
             tc.tile_pool(name="ps", bufs=1, space="PSUM") as ps:
            # scalar-engine warmup: trigger the exp table load early
            warm = pp.tile([1, 8], F32, tag="warm", name="warm")
            nc.gpsimd.memset(warm, 1.0)
            nc.scalar.activation(warm, warm, mybir.ActivationFunctionType.Ln)
            nc.scalar.activation(warm, warm, Exp)

            # ---- input DMAs (spread across idle engine queues) ----
            bqk_sb = pp.tile([CHUNK, 4], F32, tag="bqk", name="bqk_sb")
            bv_bc = pp.tile([CHUNK, DPC], F32, tag="bv_bc", name="bv_bc")

            def load_x8(strip, parts, engs):
                """Load fp8 x strip (all NCP c-pairs) into one wide tile."""
                xts = wp.tile([CHUNK, XW], FP8, tag="x8s", bufs=2,
                              name=f"x8s_{strip}")
                c0 = 0
                for i, n in enumerate(parts):
                    engs[i % len(engs)].dma_start(
                        out=xts[:, c0 * 2 * SST:(c0 + n) * 2 * SST].rearrange(
                            "p (c i t) -> p c i t", i=2, t=SST),
                        in_=x8[:, c0:c0 + n, :,
                               strip * SST:(strip + 1) * SST])
                    c0 += n
                return xts

            def load_xb(strip, parts, engs):
                """Load bf16 x strip row-chunks (for the v projection)."""
                xbs = wp.tile([CHUNK, n_cchunk * SST], BF16, tag="xbs", bufs=2,
                              name=f"xbs_{strip}")
                c0 = 0
                for i, n in enumerate(parts):
                    engs[i % len(engs)].dma_start(
                        out=xbs[:, c0 * SST:(c0 + n) * SST].rearrange(
                            "p (c t) -> p c t", t=SST),
                        in_=xT[c0 * CHUNK:(c0 + n) * CHUNK,
                               strip * SST:(strip + 1) * SST].rearrange(
                            "(c p) t -> p c t", p=CHUNK))
                    c0 += n
                return xbs

            nc.gpsimd.dma_start(out=bqk_sb, in_=bqk[:, :])
            nc.gpsimd.dma_start(out=bv_bc, in_=bv_row[0:1, :].broadcast_to((CHUNK, DPC)))
            wts = pp.tile([CHUNK, WW], FP8, tag="wts", name="wts")
            wvb = pp.tile([CHUNK, n_cchunk * DPC], BF16, tag="wvb", name="wvb")
            x8s0 = wp.tile([CHUNK, XW], FP8, tag="x8s", bufs=2, name="x8s_0")
            rr = [nc.gpsimd, nc.sync, nc.scalar]
            qi = 0
            for c in range(NCP):
                rr[qi % 3].dma_start(
                    out=wts[:, c * 2 * 2 * DPC:(c + 1) * 2 * 2 * DPC],
                    in_=w8[:, c, :, :])
                qi += 1
                rr[qi % 3].dma_start(
                    out=x8s0[:, c * 2 * SST:(c + 1) * 2 * SST].rearrange(
                        "p (i t) -> p i t", i=2),
                    in_=x8[:, c, :, 0:SST])
                qi += 1
            nc.gpsimd.dma_start(
                out=wvb.rearrange("p (c w) -> p c w", w=DPC),
                in_=wvT[:, :].rearrange("(c p) w -> p c w", p=CHUNK))
            xb0 = load_xb(0, [4, 4], [nc.sync, nc.scalar])
            wtv = wts.rearrange("p (c i w) -> p c i w", i=2, w=2 * DPC)
            wv_sb = [wvb[:, c * DPC:(c + 1) * DPC] for c in range(n_cchunk)]

            def xtv(xts):
                return xts.rearrange("p (c i t) -> p c i t", i=2, t=SST)

            x8_bufs = {0: x8s0}
            xb_bufs = {0: xb0}
            if nks > 1:
                x8_bufs[1] = load_x8(1, [2, 2], [nc.sync, nc.scalar])
                xb_bufs[1] = load_xb(1, [4, 4], [nc.sync, nc.scalar])

            wot = pp.tile([CHUNK, 2 * C], BF16, tag="wot", name="wot")
            wo_sb = [wot[:, 0:C], wot[:, C:2 * C]]

            def load_wo():
                nc.gpsimd.dma_start(
                    out=wot.rearrange("p (c w) -> p c w", w=C),
                    in_=woT[:, :].rearrange("(c p) w -> p c w", p=CHUNK))

            # HAM warm-up: keep the PE busy on throwaway matmuls while the
            # input DMAs land, so real work starts at 2.4 GHz
            dum = pp.tile([CHUNK, SST], BF16, tag="dum", name="dum")
            nc.vector.memset(dum, 0.0)
            for i in range(10):
                pd = ps.tile([CHUNK, SST], F32, tag="fill", bufs=2,
                             name=f"pd_{i}")
                nc.tensor.matmul(pd, dum[:, 0:CHUNK], dum, start=True, stop=True)

            # ---- persistent activations ----
            # qT2/kT2[m]: heads 2m / 2m+1 stacked on partitions 0:64 / 64:128
            qT2 = [pp.tile([CHUNK, t_len], BF16, tag=f"qT{m}", name=f"qT{m}")
                   for m in range(2)]
            kT2 = [pp.tile([CHUNK, t_len], BF16, tag=f"kT{m}", name=f"kT{m}")
                   for m in range(2)]
            # strip-0 AV runs bf16 (its short rows are fp8-noise-sensitive);
            # strips >=1 run fp8 DoubleRow over s-chunk PAIRS.  Both vaug
            # forms hold [ones(=64) | 64*v] — the 64 cancels in the softmax
            # ratio (wvT/bv are scaled x64 on the host).
            vaug = [pp.tile([CHUNK, HPC * VW], BF16, tag=f"v{j}", name=f"v{j}")
                    for j in range(4)]
            for j in range(4):
                eng = nc.vector if j % 2 == 0 else nc.gpsimd
                eng.memset(vaug[j], WSC)   # ones half survives; rest overwritten
            vaug2 = [pp.tile([CHUNK, HPC * 2 * VW], FP8, tag=f"w2{u}",
                             name=f"v2{u}")
                     for u in range(n_ttile // 2)]
            for u in range(n_ttile // 2):
                eng = nc.vector if u % 2 == 0 else nc.gpsimd
                eng.memset(vaug2[u], WSC)
            aoT = [pp.tile([CHUNK, t_len], BF16, tag=f"aoT{p}", name=f"aoT{p}")
                   for p in range(2)]
            # 0/1 lower-triangle for diagonal-tile masking (DVE multiply —
            # keeps the gpsimd queue out of the per-step critical path)
            tri = pp.tile([CHUNK, CHUNK], BF16, tag="tri", name="tri")
            nc.gpsimd.memset(tri, 1.0)
            nc.gpsimd.affine_select(
                out=tri, in_=tri, pattern=[[1, CHUNK]], channel_multiplier=-1,
                base=0, compare_op=mybir.AluOpType.is_ge, fill=0.0)

            # =========== emit helpers ===========
            def proj_qk_group(strip, pj, m, xts):
                """q or k (pj=0/1) for head-pair m over one t-strip: fp8 DR."""
                xt = xtv(xts)
                pq = ps.tile([CHUNK, SST], F32, tag="fill", bufs=2,
                             name=f"pq{pj}{m}_{strip}")
                base = pj * DPC + m * CHUNK
                for cp in range(NCP):
                    nc.tensor.matmul(
                        pq,
                        wtv[:, cp, :, base:base + CHUNK],
                        xt[:, cp, :, :],
                        start=(cp == 0), stop=(cp == NCP - 1),
                        perf_mode=DR)
                dst = kT2[m] if pj else qT2[m]
                # q additionally carries the softmax 1/sqrt(64)=1/8
                sc = 1.0 / (WSC * 8.0) if pj == 0 else 1.0 / WSC
                nc.vector.tensor_scalar(
                    dst[:, strip * SST:(strip + 1) * SST], pq,
                    sc, bqk_sb[:, 2 * pj + m:2 * pj + m + 1], MUL, ADD)

            def proj_v_group(strip, u, xbs):
                """v for t-chunk 4*strip+u -> vaug (bf16 matmuls)."""
                jt = 4 * strip + u
                pv = ps.tile([CHUNK, SST], F32, tag="fill", bufs=2,
                             name=f"pv_{jt}")
                for c in range(n_cchunk):
                    nc.tensor.matmul(
                        pv[:, 0:DPC],
                        xbs[:, c * SST + u * CHUNK:c * SST + (u + 1) * CHUNK],
                        wv_sb[c],
                        start=(c == 0), stop=(c == n_cchunk - 1))
                u2, i2 = jt // 2, jt % 2
                nc.vector.tensor_add(
                    vaug2[u2].rearrange("p (h i e) -> p h i e", i=2, e=VW)
                    [:, :, i2, D:2 * D],
                    pv[:, 0:DPC].rearrange("p (h d) -> p h d", d=D),
                    bv_bc.rearrange("p (h d) -> p h d", d=D))
                if jt < 4:   # strip-0 also needs the bf16 copy
                    nc.vector.tensor_add(
                        vaug[jt].rearrange("p (h e) -> p h e", e=VW)
                        [:, :, D:2 * D],
                        pv[:, 0:DPC].rearrange("p (h d) -> p h d", d=D),
                        bv_bc.rearrange("p (h d) -> p h d", d=D))

            def outproj_group(jt, tail=False):
                for js in range(2):
                    py = ps.tile([CHUNK, SST], F32, tag="fill", bufs=2,
                                 name=f"py_{jt}_{js}")
                    for p in range(2):
                        nc.tensor.matmul(
                            py,
                            aoT[p][:, jt * CHUNK:(jt + 1) * CHUNK],
                            wo_sb[p][:, js * SST:(js + 1) * SST],
                            start=(p == 0), stop=(p == 1))
                    ysb = wp.tile([CHUNK, SST], BF16, tag="ysb", bufs=3,
                                  name=f"ysb_{jt}_{js}")
                    if tail and (2 * jt + js) % 2 == 0:
                        # the scalar engine is idle in the tail; splitting the
                        # PSUM evictions halves the vector-bound epilogue
                        nc.scalar.activation(ysb, py,
                                             mybir.ActivationFunctionType.Copy)
                    else:
                        nc.vector.tensor_copy(ysb, py)
                    if tail:
                        eng = [nc.sync, nc.scalar][(2 * jt + js) % 2]
                    else:
                        eng = [nc.gpsimd, nc.sync][(2 * jt + js) % 2]
                    eng.dma_start(
                        out=y[jt * CHUNK:(jt + 1) * CHUNK,
                              js * SST:(js + 1) * SST],
                        in_=ysb)

            pend = {}
            epend = {}

            def scores_step(p, ks, j):
                """scores -> exp -> (mask) for one s-chunk j of strip ks.

                Scores for heads 2p / 2p+1 run concurrently via PE row tiling
                (kT2/qT2 partition halves); head B in its own PSUM bank.
                One strided exp covers both banks.  For strips >= 1 the exp
                writes fp8 into the shared chunk-PAIR tile ePair (layout
                [i(chunk) | head | t]) consumed later by a DoubleRow AV; the
                pair shares the even chunk's t-window, with the odd chunk's
                leading CHUNK columns masked off.  Strip 0 keeps the bf16
                per-chunk path.
                """
                paired = ks >= 1
                jb = (j - (j % 2)) if paired else j
                off = max(0, CHUNK * jb - SST * ks)
                L = SST - off
                t0 = SST * ks + off
                jc = j * CHUNK
                diag = CHUNK * jb >= SST * ks
                sAB = ps.tile([CHUNK, 2 * SST], F32, tag="sAB", bufs=2,
                              name=f"s_{p}_{ks}_{j}")
                nc.tensor.matmul(
                    sAB[:, 0:L],
                    kT2[p][0:D, jc:jc + CHUNK],
                    qT2[p][0:D, t0:t0 + L],
                    start=True, stop=True, skip_group_check=True)
                nc.tensor.matmul(
                    sAB[:, SST:SST + L],
                    kT2[p][D:CHUNK, jc:jc + CHUNK],
                    qT2[p][D:CHUNK, t0:t0 + L],
                    start=True, stop=True, skip_group_check=True)
                s_in = sAB.rearrange("p (c t) -> p c t", t=SST)[:, :, 0:L]
                if not paired:
                    eAB = wp.tile([CHUNK, 2 * SST], BF16, tag="eAB", bufs=3,
                                  name=f"e_{p}_{ks}_{j}")
                    if L == SST:
                        nc.scalar.activation(eAB, sAB, Exp)
                    else:
                        nc.scalar.activation(
                            eAB[:, 0:2 * L].rearrange(
                                "p (c t) -> p c t", c=2), s_in, Exp)
                    if diag:
                        for base in (0, L):
                            nc.vector.tensor_mul(
                                eAB[:, base:base + CHUNK],
                                eAB[:, base:base + CHUNK], tri)
                    pend[j] = (eAB, off, L)
                    return
                i = j % 2
                if i == 0:
                    eP = wp.tile([CHUNK, 4 * SST], FP8, tag="eP", bufs=3,
                                 name=f"eP_{p}_{ks}_{j}")
                    epend[j // 2] = (eP, off, L)
                else:
                    eP = epend[j // 2][0]
                ePv = eP.rearrange("p (i c t) -> p i c t", i=2, c=2)
                nc.scalar.activation(ePv[:, i, :, 0:L], s_in, Exp)
                if diag:
                    if i == 0:
                        for c in range(2):
                            nc.vector.tensor_mul(
                                ePv[:, 0, c, 0:CHUNK],
                                ePv[:, 0, c, 0:CHUNK], tri)
                    else:
                        for c in range(2):
                            nc.vector.memset(ePv[:, 1, c, 0:CHUNK], 0.0)
                            nc.vector.tensor_mul(
                                ePv[:, 1, c, CHUNK:2 * CHUNK],
                                ePv[:, 1, c, CHUNK:2 * CHUNK], tri)

            def av_step(p, ks, j, nj):
                hA, hB = 2 * p, 2 * p + 1
                eAB, off, L = pend.pop(j)
                av = av_cur[0]
                nc.tensor.matmul(
                    av[:, off:SST],
                    vaug[j][:, hA * VW:(hA + 1) * VW],
                    eAB[:, 0:L],
                    start=(j == 0), stop=(j == nj - 1), skip_group_check=True)
                nc.tensor.matmul(
                    av[:, SST + off:2 * SST],
                    vaug[j][:, hB * VW:(hB + 1) * VW],
                    eAB[:, L:2 * L],
                    start=(j == 0), stop=(j == nj - 1), skip_group_check=True)

            def av_pair(p, ks, u, nu, pe):
                """fp8 DoubleRow AV over s-chunk pair u (strips >= 1)."""
                eP, off, L = pe
                av = av_cur[0]
                ePv = eP.rearrange("p (i c t) -> p i c t", i=2, c=2)
                for c in range(2):
                    nc.tensor.matmul(
                        av[:, c * SST + off:(c + 1) * SST],
                        vaug2[u][:, (2 * p + c) * 2 * VW:
                                 (2 * p + c + 1) * 2 * VW]
                        .rearrange("p (i e) -> p i e", i=2),
                        ePv[:, :, c, 0:L],
                        start=(u == 0), stop=(u == nu - 1),
                        perf_mode=DR, skip_group_check=True)

            pending_mul = []

            def flush_mul():
                while pending_mul:
                    p, ks, bcf = pending_mul.pop(0)
                    strip = slice(ks * SST, (ks + 1) * SST)
                    half = SST // 2
                    lo = ks * SST
                    nc.gpsimd.tensor_mul(aoT[p][:, lo:lo + half],
                                         aoT[p][:, lo:lo + half],
                                         bcf[:, 0:half])
                    nc.gpsimd.tensor_mul(aoT[p][:, lo + half:lo + SST],
                                         aoT[p][:, lo + half:lo + SST],
                                         bcf[:, half:SST])

            def normalize(p, ks, tail=False):
                """softmax-normalize strip ks of pair p into aoT[p].

                av rows 0:64 hold the PE-replicated denominators, rows 64:128
                the unnormalized outputs.  Reciprocals run directly on the
                PSUM denominator rows (the DVE 32-lane shuffle crossbar
                handles the 64-partition shift), no DRAM bounce needed.  The
                final aoT multiply is deferred (flush_mul) so it never blocks
                the next pair's masks on the gpsimd queue — except in the
                tail, where it runs immediately."""
                av = av_cur[0]
                strip = slice(ks * SST, (ks + 1) * SST)
                half = SST // 2
                lo = ks * SST
                if tail:
                    # low-latency path: 1/d = exp(-log d) on the (idle) scalar
                    # (Ln of the A-half reads the PSUM denominator directly;
                    # the B-half needs the DVE crossbar for the 64-partition
                    # shift, so it goes through a copy)
                    rec = wp.tile([CHUNK, SST], F32, tag="rec", bufs=2,
                                  name=f"rec_{p}_{ks}")
                    nc.scalar.activation(rec[0:D, :], av[0:D, 0:SST],
                                         mybir.ActivationFunctionType.Ln)
                    nc.vector.tensor_copy(rec[D:CHUNK, :], av[0:D, SST:2 * SST])
                    nc.vector.tensor_copy(aoT[p][0:D, strip],
                                          av[D:CHUNK, 0:SST])
                    nc.scalar.activation(rec[D:CHUNK, :], rec[D:CHUNK, :],
                                         mybir.ActivationFunctionType.Ln)
                    nc.vector.tensor_copy(aoT[p][D:CHUNK, strip],
                                          av[D:CHUNK, SST:2 * SST])
                    nc.scalar.activation(rec, rec, Exp, scale=-1.0)
                    nc.gpsimd.tensor_mul(aoT[p][:, lo:lo + half],
                                         aoT[p][:, lo:lo + half],
                                         rec[:, 0:half])
                    nc.vector.tensor_mul(aoT[p][:, lo + half:lo + SST],
                                         aoT[p][:, lo + half:lo + SST],
                                         rec[:, half:SST])
                    return
                den = wp.tile([1, 2 * SST], F32, tag="den", bufs=2,
                              name=f"den_{p}_{ks}")
                # evicts; frees the av banks.  B-side evict on the scalar
                # engine (idle at pair boundaries).
                nc.vector.tensor_copy(aoT[p][0:D, strip],
                                      av[D:CHUNK, 0:SST])
                nc.scalar.activation(aoT[p][D:CHUNK, strip],
                                     av[D:CHUNK, SST:2 * SST],
                                     mybir.ActivationFunctionType.Copy)
                nc.vector.tensor_copy(den[0:1, 0:SST], av[0:1, 0:SST])
                nc.vector.tensor_copy(den[0:1, SST:2 * SST],
                                      av[0:1, SST:2 * SST])
                # reciprocal via a [128, 8] reshape (wide on the DVE lanes),
                # then partition-broadcast — both through small DRAM bounces
                # (SBUF APs cannot reshape across partitions / stride-0 bcast).
                # Latency is hidden: the aoT multiply is deferred into the
                # next pair's j-loop (flush_mul).
                dden = dr.tile([1, 2 * SST], F32, tag="dden", bufs=2,
                               name=f"dden_{p}_{ks}")
                nc.sync.dma_start(out=dden, in_=den)
                nf = 2 * SST // CHUNK
                dsb = wp.tile([CHUNK, nf], F32, tag="dsb", bufs=2,
                              name=f"dsb_{p}_{ks}")
                dview = dden.rearrange("a b -> (a b)").rearrange(
                    "(p f) -> p f", p=CHUNK)
                nc.sync.dma_start(out=dsb, in_=dview)
                rsb = wp.tile([CHUNK, nf], F32, tag="rsb", bufs=2,
                              name=f"rsb_{p}_{ks}")
                nc.vector.reciprocal(rsb, dsb)
                drec = dr.tile([1, 2 * SST], F32, tag="drec", bufs=2,
                               name=f"drec_{p}_{ks}")
                rview = drec.rearrange("a b -> (a b)").rearrange(
                    "(p f) -> p f", p=CHUNK)
                nc.sync.dma_start(out=rview, in_=rsb)
                bcf = wp.tile([CHUNK, SST], F32, tag="bcf", bufs=2,
                              name=f"bcf_{p}_{ks}")
                nc.sync.dma_start(
                    out=bcf[0:D, :],
                    in_=drec[0:1, 0:SST].broadcast_to((D, SST)))
                nc.sync.dma_start(
                    out=bcf[D:CHUNK, :],
                    in_=drec[0:1, SST:2 * SST].broadcast_to((D, SST)))
                pending_mul.append((p, ks, bcf))

            # =========== schedule ===========
            # pre-loop: full projection of strip 0
            for pj in (1, 0):
                for m in range(2):
                    proj_qk_group(0, pj, m, x8_bufs[0])
            for u in range(4):
                proj_v_group(0, u, xb_bufs[0])

            av_cur = [None]
            for ks in range(nks):
                if ks == 0:
                    load_wo()
                # prefetch x strip ks+2 (its buffers were freed by proj(ks))
                if ks + 2 < nks:
                    x8_bufs[ks + 2] = load_x8(ks + 2, [2, 2], [nc.scalar])
                    xb_bufs[ks + 2] = load_xb(ks + 2, [4, 4], [nc.scalar])

                fillers = []
                if ks + 1 < nks:                  # projection of next strip
                    for pj in (1, 0):
                        for m in range(2):
                            fillers.append(
                                (proj_qk_group, (ks + 1, pj, m, x8_bufs[ks + 1])))
                    for u in range(4):
                        fillers.append(
                            (proj_v_group, (ks + 1, u, xb_bufs[ks + 1])))
                if ks == nks - 1:                 # deferred out-proj
                    for jt in range(4 * (nks - 1)):
                        fillers.append((outproj_group, (jt,)))

                nj = 4 * ks + 4
                total_js = 2 * nj
                gi = 0
                cnt = 0
                for p in (0, 1):
                    av_cur[0] = ps.tile([CHUNK, 2 * SST], F32, tag="av", bufs=1,
                                        name=f"av_{p}_{ks}")
                    res = (4 if p == 0 else 2) if ks == nks - 1 else 2
                    cap = max(0, len(fillers) - res)
                    paired = ks >= 1
                    nu = nj // 2
                    for j in range(nj):
                        scores_step(p, ks, j)
                        if paired:
                            if j >= 3 and j % 2 == 1:
                                u = (j - 3) // 2
                                av_pair(p, ks, u, nu, epend.pop(u))
                        elif j > 1:
                            av_step(p, ks, j - 2, nj)
                        if j == 3:
                            flush_mul()   # previous pair's deferred aoT mul
                        cnt += 1
                        want = min(cap,
                                   ((cnt + 2) * len(fillers)) // total_js)
                        while gi < want:
                            fn, args = fillers[gi]
                            fn(*args)
                            gi += 1
                    if paired:
                        av_pair(p, ks, nu - 1, nu, epend.pop(nu - 1))
                    else:
                        av_step(p, ks, nj - 2, nj)
                        av_step(p, ks, nj - 1, nj)
                    normalize(p, ks, tail=(ks == nks - 1 and p == 1))
                    # keep the tensor queue fed while av drains
                    for _ in range(res):
                        if gi < len(fillers):
                            fn, args = fillers[gi]
                            fn(*args)
                            gi += 1
                while gi < len(fillers):
                    fn, args = fillers[gi]
                    fn(*args)
                    gi += 1

            # tail: out-proj of the last strip
            for jt in range(4 * (nks - 1), 4 * nks):
                outproj_group(jt, tail=True)

    if fix_waits:
        split_excess_waits(nc)
    return nc


def make_in_maps(x, wq, bq, wk, bk, wv, bv, wo, bo, t_len=T):
    """Build the 8 per-core input dicts from full inputs."""
    in_maps = []
    NCP = C // 256

    for core in range(NCORES):
        b, hg = core // 4, core % 4
        sl = slice(DPC * hg, DPC * (hg + 1))
        wqk = np.concatenate(
            [wq[sl].T, wk[sl].T], axis=1).astype(np.float32) * WSC
        w8 = np.ascontiguousarray(
            wqk.reshape(NCP, 2, CHUNK, -1).transpose(2, 0, 1, 3)).astype(F8_NP)
        bqs = (bq[sl] / 8.0).astype(np.float32)
        bqkm = np.stack([bqs[0:CHUNK], bqs[CHUNK:2 * CHUNK],
                         bk[sl][0:CHUNK], bk[sl][CHUNK:2 * CHUNK]], axis=1)
        xTb = np.ascontiguousarray(x[b, :t_len].T)
        x8r = np.ascontiguousarray(
            xTb.reshape(NCP, 2, CHUNK, t_len).transpose(2, 0, 1, 3)
        ).astype(F8_NP)
        in_maps.append({
            "x8": x8r,
            "w8": w8,
            "xT": xTb.astype(BF16_NP),
            "wvT": np.ascontiguousarray(wv[sl].T * WSC).astype(BF16_NP),
            "woT": np.ascontiguousarray(wo[:, sl].T).astype(BF16_NP),
            "bqk": np.ascontiguousarray(bqkm, dtype=np.float32),
            "bv_row": np.ascontiguousarray(
                (bv[sl] * WSC)[None, :], dtype=np.float32),
        })
    return in_maps


def gather_output(results, bo, t_len=T):
    ys = [np.asarray(results[i]["y"], dtype=np.float32) for i in range(NCORES)]
    out = np.stack([ys[0] + ys[1] + ys[2] + ys[3],
                    ys[4] + ys[5] + ys[6] + ys[7]])
    out += np.asarray(bo, np.float32)[None, None, :]
    return out


_NC_CACHE = {}


def _get_nc(t_len=T):
    if t_len not in _NC_CACHE:
        _NC_CACHE[t_len] = build_nc(t_len)
    return _NC_CACHE[t_len]


def kernel(x, wq, bq, wk, bk, wv, bv, wo, bo, mask=None, **_unused):
    """Full-input entry point: shard, run on 8 NeuronCores, gather."""
    from concourse.bass_utils import run_bass_kernel_spmd

    x = np.asarray(x, dtype=np.float32)
    in_maps = make_in_maps(x, np.asarray(wq, np.float32), np.asarray(bq, np.float32),
                           np.asarray(wk, np.float32), np.asarray(bk, np.float32),
                           np.asarray(wv, np.float32), np.asarray(bv, np.float32),
                           np.asarray(wo, np.float32), np.asarray(bo, np.float32))
    nc = _get_nc(T)
    res = run_bass_kernel_spmd(nc, in_maps, list(range(NCORES)))
    return gather_output(res.results, bo)


# revision 30
# speedup vs baseline: 1.0543x; 1.0131x over previous
"""Trainium2 Bass kernel: causal multi-head attention block (B=2, T=2048, C=1024, H=16).

Sharding: 8 cores = 2 (batch) x 4 (head groups of 4 heads).  Each core computes
q/k/v projections for its 4 heads, causal attention, and a partial out-proj
(rows of wo for its head slice).  Host sums the 4 partials per batch element.

v6: v3's software-pipelined schedule plus two PE packs, with precision kept
where fp8 noise does not average out:
  - q/k projections in fp8e4 with perf_mode=DoubleRow (contraction pairs
    packed 2-per-PE-cell: 4 matmuls per 1024-deep projection instead of 8).
    Weights scaled x64 on the host (fp8e4 subnormal cutoff 2^-6); the 1/64
    rides the existing bias-add.  Softmax is insensitive to the ~4% fp8
    element noise on scores (row-common factors cancel; diffuse rows
    average), unlike the v/out paths where fp8 noise lands directly on the
    output (measured ~5e-2 rel err) — so v-proj and out-proj stay bf16.
  - scores via PE row tiling: head A in PE rows 0:64, head B in rows 64:128
    (qT/kT stored as stacked pair tiles); the two score matmuls issue
    back-to-back and run concurrently in the array (~2x).  Head B keeps its
    own PSUM bank (same-bank packing + concurrency hangs the NEFF).
  - one strided exp activation per j-step ([A|B] banks in a single 3D AP);
    Act engine runs exp only, copies live on DVE.

Per-core layouts:
  x8      [128, 4, 2, 2048]  x[b].T partition-major c-pairs      (fp8e4)
  w8      [128, 4, 2, 512]   [64*wq_s.T | 64*wk_s.T] pairs       (fp8e4)
  xT      [1024, 2048]       x[b].T                              (bf16)
  wvT     [1024, 256]        wv_s.T                              (bf16)
  woT     [256, 1024]        wo[:, head_slice].T                 (bf16)
  bqk     [128, 4]  cols: bq/8 (pair0,pair1), bk (pair0,pair1)   (f32)
  bv_row  [1, 256]  bv                                           (f32)
  y       [2048, 1024]  partial output (pre-sum, pre-bo)         (bf16)
"""

import os
import sys

import numpy as np
import ml_dtypes

F8_NP = ml_dtypes.float8_e4m3   # TRN fp8e4: max 240, inf at 256
BF16_NP = ml_dtypes.bfloat16

for _p in ("/opt/trn_rl_repo", "/root/.axon_site/_ro/trn_rl_repo"):
    if os.path.isdir(_p) and _p not in sys.path:
        sys.path.append(_p)

import concourse.bass as bass  # noqa: E402
import concourse.mybir as mybir  # noqa: E402
import concourse.tile as tile  # noqa: E402

F32 = mybir.dt.float32
BF16 = mybir.dt.bfloat16
FP8 = mybir.dt.float8e4
DR = mybir.MatmulPerfMode.DoubleRow
MUL = mybir.AluOpType.mult
ADD = mybir.AluOpType.add

B, T, C, H = 2, 2048, 1024, 16
D = C // H          # 64
HPC = 4             # heads per core
DPC = HPC * D       # 256 head-dims per core
NCORES = 8

CHUNK = 128         # s-chunk / contraction granularity
SST = 512           # attention t-strip == one PSUM bank of f32
VW = 2 * D          # 128 per head: [ones x64 | v x64]
WSC = 64.0          # host-side fp8 weight scale

_CTRL_TYPES = (mybir.InstDrain, mybir.InstNoOp, mybir.InstEventSemaphore)


def split_excess_waits(nc, lim=1):
    """Walrus accepts at most one sync-wait per instruction; move extras onto
    same-engine NoOps inserted just before the owner."""
    k = 0
    for fn in nc.m.functions:
        for blk in fn.blocks:
            out = []
            changed = False
            for inst in blk.instructions:
                si = inst.sync_info
                if si is not None and si.on_wait and len(si.on_wait) > lim:
                    waits = list(si.on_wait)
                    extra, keep = waits[:-lim], waits[-lim:]
                    for w in extra:
                        nop = mybir.InstNoOp(name=f"waitfix_{k}", ins=[], outs=[])
                        k += 1
                        nop.engine = inst.engine
                        nop.sync_info = mybir.SyncInfo(on_wait=[w], on_update=[])
                        out.append(nop)
                    si.on_wait = keep
                    changed = True
                out.append(inst)
            if changed:
                blk.instructions = out
    return k


def build_nc(t_len=T, fix_waits=True):
    """Build the per-core SPMD Bass program (same program on all 8 cores)."""
    assert t_len % SST == 0
    nks = t_len // SST                # 4 strips
    n_cchunk = C // CHUNK             # 8
    n_ttile = t_len // CHUNK          # 16
    NCP = C // 256                    # 4 contraction pairs

    nc = bass.Bass(target_bir_lowering=False)

    x8 = nc.dram_tensor("x8", [CHUNK, NCP, 2, t_len], FP8, kind="ExternalInput")
    w8 = nc.dram_tensor("w8", [CHUNK, NCP, 2, 2 * DPC], FP8, kind="ExternalInput")
    xT = nc.dram_tensor("xT", [C, t_len], BF16, kind="ExternalInput")
    wvT = nc.dram_tensor("wvT", [C, DPC], BF16, kind="ExternalInput")
    woT = nc.dram_tensor("woT", [DPC, C], BF16, kind="ExternalInput")
    bqk = nc.dram_tensor("bqk", [CHUNK, 4], F32, kind="ExternalInput")
    bv_row = nc.dram_tensor("bv_row", [1, DPC], F32, kind="ExternalInput")
    y = nc.dram_tensor("y", [t_len, C], BF16, kind="ExternalOutput")

    Exp = mybir.ActivationFunctionType.Exp
    XW = 2 * NCP * SST               # per-strip fp8 x tile cols
    WW = 2 * NCP * 2 * DPC

    with tile.TileContext(nc) as tc:
        with tc.tile_pool(name="persist", bufs=1) as pp, \
             tc.tile_pool(name="work", bufs=1) as wp, \
             tc.tile_pool(name="dr", bufs=1, space="DRAM") as dr, \
             tc.tile_pool(name="ps", bufs=1, space="PSUM") as ps:
            # scalar-engine warmup: trigger the exp table load early
            warm = pp.tile([1, 8], F32, tag="warm", name="warm")
            nc.gpsimd.memset(warm, 1.0)
            nc.scalar.activation(warm, warm, mybir.ActivationFunctionType.Ln)
            nc.scalar.activation(warm, warm, Exp)

            # ---- input DMAs (spread across idle engine queues) ----
            bqk_sb = pp.tile([CHUNK, 4], F32, tag="bqk", name="bqk_sb")
            bv_bc = pp.tile([CHUNK, DPC], F32, tag="bv_bc", name="bv_bc")

            def load_x8(strip, parts, engs):
                """Load fp8 x strip (all NCP c-pairs) into one wide tile."""
                xts = wp.tile([CHUNK, XW], FP8, tag="x8s", bufs=2,
                              name=f"x8s_{strip}")
                c0 = 0
                for i, n in enumerate(parts):
                    engs[i % len(engs)].dma_start(
                        out=xts[:, c0 * 2 * SST:(c0 + n) * 2 * SST].rearrange(
                            "p (c i t) -> p c i t", i=2, t=SST),
                        in_=x8[:, c0:c0 + n, :,
                               strip * SST:(strip + 1) * SST])
                    c0 += n
                return xts

            def load_xb(strip, parts, engs):
                """Load bf16 x strip row-chunks (for the v projection)."""
                xbs = wp.tile([CHUNK, n_cchunk * SST], BF16, tag="xbs", bufs=2,
                              name=f"xbs_{strip}")
                c0 = 0
                for i, n in enumerate(parts):
                    engs[i % len(engs)].dma_start(
                        out=xbs[:, c0 * SST:(c0 + n) * SST].rearrange(
                            "p (c t) -> p c t", t=SST),
                        in_=xT[c0 * CHUNK:(c0 + n) * CHUNK,
                               strip * SST:(strip + 1) * SST].rearrange(
                            "(c p) t -> p c t", p=CHUNK))
                    c0 += n
                return xbs

            nc.gpsimd.dma_start(out=bqk_sb, in_=bqk[:, :])
            nc.gpsimd.dma_start(out=bv_bc, in_=bv_row[0:1, :].broadcast_to((CHUNK, DPC)))
            wts = pp.tile([CHUNK, WW], FP8, tag="wts", name="wts")
            wvb = pp.tile([CHUNK, n_cchunk * DPC], BF16, tag="wvb", name="wvb")
            x8s0 = wp.tile([CHUNK, XW], FP8, tag="x8s", bufs=2, name="x8s_0")
            rr = [nc.gpsimd, nc.sync, nc.scalar]
            qi = 0
            for c in range(NCP):
                rr[qi % 3].dma_start(
                    out=wts[:, c * 2 * 2 * DPC:(c + 1) * 2 * 2 * DPC],
                    in_=w8[:, c, :, :])
                qi += 1
                rr[qi % 3].dma_start(
                    out=x8s0[:, c * 2 * SST:(c + 1) * 2 * SST].rearrange(
                        "p (i t) -> p i t", i=2),
                    in_=x8[:, c, :, 0:SST])
                qi += 1
            nc.gpsimd.dma_start(
                out=wvb.rearrange("p (c w) -> p c w", w=DPC),
                in_=wvT[:, :].rearrange("(c p) w -> p c w", p=CHUNK))
            xb0 = load_xb(0, [4, 4], [nc.sync, nc.scalar])
            wtv = wts.rearrange("p (c i w) -> p c i w", i=2, w=2 * DPC)
            wv_sb = [wvb[:, c * DPC:(c + 1) * DPC] for c in range(n_cchunk)]

            def xtv(xts):
                return xts.rearrange("p (c i t) -> p c i t", i=2, t=SST)

            x8_bufs = {0: x8s0}
            xb_bufs = {0: xb0}
            if nks > 1:
                x8_bufs[1] = load_x8(1, [2, 2], [nc.sync, nc.scalar])
                xb_bufs[1] = load_xb(1, [4, 4], [nc.sync, nc.scalar])

            wot = pp.tile([CHUNK, 2 * C], BF16, tag="wot", name="wot")
            wo_sb = [wot[:, 0:C], wot[:, C:2 * C]]

            def load_wo():
                nc.gpsimd.dma_start(
                    out=wot.rearrange("p (c w) -> p c w", w=C),
                    in_=woT[:, :].rearrange("(c p) w -> p c w", p=CHUNK))

            # HAM warm-up: keep the PE busy on throwaway matmuls while the
            # input DMAs land, so real work starts at 2.4 GHz
            dum = pp.tile([CHUNK, SST], BF16, tag="dum", name="dum")
            nc.vector.memset(dum, 0.0)
            for i in range(10):
                pd = ps.tile([CHUNK, SST], F32, tag="fill", bufs=2,
                             name=f"pd_{i}")
                nc.tensor.matmul(pd, dum[:, 0:CHUNK], dum, start=True, stop=True)

            # ---- persistent activations ----
            # qT2/kT2[m]: heads 2m / 2m+1 stacked on partitions 0:64 / 64:128
            qT2 = [pp.tile([CHUNK, t_len], BF16, tag=f"qT{m}", name=f"qT{m}")
                   for m in range(2)]
            kT2 = [pp.tile([CHUNK, t_len], BF16, tag=f"kT{m}", name=f"kT{m}")
                   for m in range(2)]
            # strip-0 AV runs bf16 (its short rows are fp8-noise-sensitive);
            # strips >=1 run fp8 DoubleRow over s-chunk PAIRS.  Both vaug
            # forms hold [ones(=64) | 64*v] — the 64 cancels in the softmax
            # ratio (wvT/bv are scaled x64 on the host).
            vaug = [pp.tile([CHUNK, HPC * VW], BF16, tag=f"v{j}", name=f"v{j}")
                    for j in range(4)]
            for j in range(4):
                eng = nc.vector if j % 2 == 0 else nc.gpsimd
                eng.memset(vaug[j], WSC)   # ones half survives; rest overwritten
            vaug2 = [pp.tile([CHUNK, HPC * 2 * VW], FP8, tag=f"w2{u}",
                             name=f"v2{u}")
                     for u in range(n_ttile // 2)]
            for u in range(n_ttile // 2):
                eng = nc.vector if u % 2 == 0 else nc.gpsimd
                eng.memset(vaug2[u], WSC)
            aoT = [pp.tile([CHUNK, t_len], BF16, tag=f"aoT{p}", name=f"aoT{p}")
                   for p in range(2)]
            # 0/1 lower-triangle for diagonal-tile masking (DVE multiply —
            # keeps the gpsimd queue out of the per-step critical path)
            tri = pp.tile([CHUNK, CHUNK], BF16, tag="tri", name="tri")
            nc.gpsimd.memset(tri, 1.0)
            nc.gpsimd.affine_select(
                out=tri, in_=tri, pattern=[[1, CHUNK]], channel_multiplier=-1,
                base=0, compare_op=mybir.AluOpType.is_ge, fill=0.0)

            # =========== emit helpers ===========
            def proj_qk_group(strip, pj, m, xts):
                """q or k (pj=0/1) for head-pair m over one t-strip: fp8 DR."""
                xt = xtv(xts)
                pq = ps.tile([CHUNK, SST], F32, tag="fill", bufs=2,
                             name=f"pq{pj}{m}_{strip}")
                base = pj * DPC + m * CHUNK
                for cp in range(NCP):
                    nc.tensor.matmul(
                        pq,
                        wtv[:, cp, :, base:base + CHUNK],
                        xt[:, cp, :, :],
                        start=(cp == 0), stop=(cp == NCP - 1),
                        perf_mode=DR)
                dst = kT2[m] if pj else qT2[m]
                # q additionally carries the softmax 1/sqrt(64)=1/8
                sc = 1.0 / (WSC * 8.0) if pj == 0 else 1.0 / WSC
                nc.vector.tensor_scalar(
                    dst[:, strip * SST:(strip + 1) * SST], pq,
                    sc, bqk_sb[:, 2 * pj + m:2 * pj + m + 1], MUL, ADD)

            def proj_v_group(strip, u, xbs):
                """v for t-chunk 4*strip+u -> vaug (bf16 matmuls)."""
                jt = 4 * strip + u
                pv = ps.tile([CHUNK, SST], F32, tag="fill", bufs=2,
                             name=f"pv_{jt}")
                for c in range(n_cchunk):
                    nc.tensor.matmul(
                        pv[:, 0:DPC],
                        xbs[:, c * SST + u * CHUNK:c * SST + (u + 1) * CHUNK],
                        wv_sb[c],
                        start=(c == 0), stop=(c == n_cchunk - 1))
                u2, i2 = jt // 2, jt % 2
                nc.vector.tensor_add(
                    vaug2[u2].rearrange("p (h i e) -> p h i e", i=2, e=VW)
                    [:, :, i2, D:2 * D],
                    pv[:, 0:DPC].rearrange("p (h d) -> p h d", d=D),
                    bv_bc.rearrange("p (h d) -> p h d", d=D))
                if jt < 4:   # strip-0 also needs the bf16 copy
                    nc.vector.tensor_add(
                        vaug[jt].rearrange("p (h e) -> p h e", e=VW)
                        [:, :, D:2 * D],
                        pv[:, 0:DPC].rearrange("p (h d) -> p h d", d=D),
                        bv_bc.rearrange("p (h d) -> p h d", d=D))

            def outproj_group(jt, tail=False):
                for js in range(2):
                    py = ps.tile([CHUNK, SST], F32, tag="fill", bufs=2,
                                 name=f"py_{jt}_{js}")
                    for p in range(2):
                        nc.tensor.matmul(
                            py,
                            aoT[p][:, jt * CHUNK:(jt + 1) * CHUNK],
                            wo_sb[p][:, js * SST:(js + 1) * SST],
                            start=(p == 0), stop=(p == 1))
                    ysb = wp.tile([CHUNK, SST], BF16, tag="ysb", bufs=3,
                                  name=f"ysb_{jt}_{js}")
                    if tail and (2 * jt + js) % 2 == 0:
                        # the scalar engine is idle in the tail; splitting the
                        # PSUM evictions halves the vector-bound epilogue
                        nc.scalar.activation(ysb, py,
                                             mybir.ActivationFunctionType.Copy)
                    else:
                        nc.vector.tensor_copy(ysb, py)
                    if tail:
                        eng = [nc.sync, nc.scalar][(2 * jt + js) % 2]
                    else:
                        eng = [nc.gpsimd, nc.sync][(2 * jt + js) % 2]
                    eng.dma_start(
                        out=y[jt * CHUNK:(jt + 1) * CHUNK,
                              js * SST:(js + 1) * SST],
                        in_=ysb)

            pend = {}
            epend = {}

            def scores_step(p, ks, j):
                """scores -> exp -> (mask) for one s-chunk j of strip ks.

                Scores for heads 2p / 2p+1 run concurrently via PE row tiling
                (kT2/qT2 partition halves); head B in its own PSUM bank.
                One strided exp covers both banks.  For strips >= 1 the exp
                writes fp8 into the shared chunk-PAIR tile ePair (layout
                [i(chunk) | head | t]) consumed later by a DoubleRow AV; the
                pair shares the even chunk's t-window, with the odd chunk's
                leading CHUNK columns masked off.  Strip 0 keeps the bf16
                per-chunk path.
                """
                paired = ks >= 1
                jb = (j - (j % 2)) if paired else j
                off = max(0, CHUNK * jb - SST * ks)
                L = SST - off
                t0 = SST * ks + off
                jc = j * CHUNK
                diag = CHUNK * jb >= SST * ks
                sAB = ps.tile([CHUNK, 2 * SST], F32, tag="sAB", bufs=2,
                              name=f"s_{p}_{ks}_{j}")
                nc.tensor.matmul(
                    sAB[:, 0:L],
                    kT2[p][0:D, jc:jc + CHUNK],
                    qT2[p][0:D, t0:t0 + L],
                    start=True, stop=True, skip_group_check=True)
                nc.tensor.matmul(
                    sAB[:, SST:SST + L],
                    kT2[p][D:CHUNK, jc:jc + CHUNK],
                    qT2[p][D:CHUNK, t0:t0 + L],
                    start=True, stop=True, skip_group_check=True)
                s_in = sAB.rearrange("p (c t) -> p c t", t=SST)[:, :, 0:L]
                if not paired:
                    eAB = wp.tile([CHUNK, 2 * SST], BF16, tag="eAB", bufs=3,
                                  name=f"e_{p}_{ks}_{j}")
                    if L == SST:
                        nc.scalar.activation(eAB, sAB, Exp)
                    else:
                        nc.scalar.activation(
                            eAB[:, 0:2 * L].rearrange(
                                "p (c t) -> p c t", c=2), s_in, Exp)
                    if diag:
                        for base in (0, L):
                            nc.vector.tensor_mul(
                                eAB[:, base:base + CHUNK],
                                eAB[:, base:base + CHUNK], tri)
                    pend[(p, j)] = (eAB, off, L)
                    return
                i = j % 2
                if i == 0:
                    eP = wp.tile([CHUNK, 4 * SST], FP8, tag="eP", bufs=3,
                                 name=f"eP_{p}_{ks}_{j}")
                    epend[(p, j // 2)] = (eP, off, L)
                else:
                    eP = epend[(p, j // 2)][0]
                ePv = eP.rearrange("p (i c t) -> p i c t", i=2, c=2)
                nc.scalar.activation(ePv[:, i, :, 0:L], s_in, Exp)
                if diag:
                    if i == 0:
                        for c in range(2):
                            nc.vector.tensor_mul(
                                ePv[:, 0, c, 0:CHUNK],
                                ePv[:, 0, c, 0:CHUNK], tri)
                    else:
                        for c in range(2):
                            nc.vector.memset(ePv[:, 1, c, 0:CHUNK], 0.0)
                            nc.vector.tensor_mul(
                                ePv[:, 1, c, CHUNK:2 * CHUNK],
                                ePv[:, 1, c, CHUNK:2 * CHUNK], tri)

            def av_step(p, ks, j, nj):
                hA, hB = 2 * p, 2 * p + 1
                eAB, off, L = pend.pop((p, j))
                av = av_cur[0]
                nc.tensor.matmul(
                    av[:, off:SST],
                    vaug[j][:, hA * VW:(hA + 1) * VW],
                    eAB[:, 0:L],
                    start=(j == 0), stop=(j == nj - 1), skip_group_check=True)
                nc.tensor.matmul(
                    av[:, SST + off:2 * SST],
                    vaug[j][:, hB * VW:(hB + 1) * VW],
                    eAB[:, L:2 * L],
                    start=(j == 0), stop=(j == nj - 1), skip_group_check=True)

            def av_pair(p, ks, u, nu, pe):
                """fp8 DoubleRow AV over s-chunk pair u (strips >= 1)."""
                eP, off, L = pe
                av = av_cur[0]
                ePv = eP.rearrange("p (i c t) -> p i c t", i=2, c=2)
                for c in range(2):
                    nc.tensor.matmul(
                        av[:, c * SST + off:(c + 1) * SST],
                        vaug2[u][:, (2 * p + c) * 2 * VW:
                                 (2 * p + c + 1) * 2 * VW]
                        .rearrange("p (i e) -> p i e", i=2),
                        ePv[:, :, c, 0:L],
                        start=(u == 0), stop=(u == nu - 1),
                        perf_mode=DR, skip_group_check=True)

            pending_mul = []

            def flush_mul():
                while pending_mul:
                    p, ks, bcf = pending_mul.pop(0)
                    strip = slice(ks * SST, (ks + 1) * SST)
                    half = SST // 2
                    lo = ks * SST
                    nc.gpsimd.tensor_mul(aoT[p][:, lo:lo + half],
                                         aoT[p][:, lo:lo + half],
                                         bcf[:, 0:half])
                    nc.gpsimd.tensor_mul(aoT[p][:, lo + half:lo + SST],
                                         aoT[p][:, lo + half:lo + SST],
                                         bcf[:, half:SST])

            def normalize(p, ks, tail=False):
                """softmax-normalize strip ks of pair p into aoT[p].

                av rows 0:64 hold the PE-replicated denominators, rows 64:128
                the unnormalized outputs.  Reciprocals run directly on the
                PSUM denominator rows (the DVE 32-lane shuffle crossbar
                handles the 64-partition shift), no DRAM bounce needed.  The
                final aoT multiply is deferred (flush_mul) so it never blocks
                the next pair's masks on the gpsimd queue — except in the
                tail, where it runs immediately."""
                av = av_cur[0]
                strip = slice(ks * SST, (ks + 1) * SST)
                half = SST // 2
                lo = ks * SST
                if tail:
                    # low-latency path: 1/d = exp(-log d) on the (idle) scalar
                    # (Ln of the A-half reads the PSUM denominator directly;
                    # the B-half needs the DVE crossbar for the 64-partition
                    # shift, so it goes through a copy)
                    rec = wp.tile([CHUNK, SST], F32, tag="rec", bufs=2,
                                  name=f"rec_{p}_{ks}")
                    nc.scalar.activation(rec[0:D, :], av[0:D, 0:SST],
                                         mybir.ActivationFunctionType.Ln)
                    nc.vector.tensor_copy(rec[D:CHUNK, :], av[0:D, SST:2 * SST])
                    nc.vector.tensor_copy(aoT[p][0:D, strip],
                                          av[D:CHUNK, 0:SST])
                    nc.scalar.activation(rec[D:CHUNK, :], rec[D:CHUNK, :],
                                         mybir.ActivationFunctionType.Ln)
                    nc.vector.tensor_copy(aoT[p][D:CHUNK, strip],
                                          av[D:CHUNK, SST:2 * SST])
                    nc.scalar.activation(rec, rec, Exp, scale=-1.0)
                    nc.gpsimd.tensor_mul(aoT[p][:, lo:lo + half],
                                         aoT[p][:, lo:lo + half],
                                         rec[:, 0:half])
                    nc.vector.tensor_mul(aoT[p][:, lo + half:lo + SST],
                                         aoT[p][:, lo + half:lo + SST],
                                         rec[:, half:SST])
                    return
                den = wp.tile([1, 2 * SST], F32, tag="den", bufs=2,
                              name=f"den_{p}_{ks}")
                # evicts; frees the av banks.  B-side evict on the scalar
                # engine (idle at pair boundaries).
                nc.vector.tensor_copy(aoT[p][0:D, strip],
                                      av[D:CHUNK, 0:SST])
                nc.scalar.activation(aoT[p][D:CHUNK, strip],
                                     av[D:CHUNK, SST:2 * SST],
                                     mybir.ActivationFunctionType.Copy)
                nc.vector.tensor_copy(den[0:1, 0:SST], av[0:1, 0:SST])
                nc.vector.tensor_copy(den[0:1, SST:2 * SST],
                                      av[0:1, SST:2 * SST])
                # reciprocal via a [128, 8] reshape (wide on the DVE lanes),
                # then partition-broadcast — both through small DRAM bounces
                # (SBUF APs cannot reshape across partitions / stride-0 bcast).
                # Latency is hidden: the aoT multiply is deferred into the
                # next pair's j-loop (flush_mul).
                dden = dr.tile([1, 2 * SST], F32, tag="dden", bufs=2,
                               name=f"dden_{p}_{ks}")
                nc.sync.dma_start(out=dden, in_=den)
                nf = 2 * SST // CHUNK
                dsb = wp.tile([CHUNK, nf], F32, tag="dsb", bufs=2,
                              name=f"dsb_{p}_{ks}")
                dview = dden.rearrange("a b -> (a b)").rearrange(
                    "(p f) -> p f", p=CHUNK)
                nc.sync.dma_start(out=dsb, in_=dview)
                rsb = wp.tile([CHUNK, nf], F32, tag="rsb", bufs=2,
                              name=f"rsb_{p}_{ks}")
                nc.vector.reciprocal(rsb, dsb)
                drec = dr.tile([1, 2 * SST], F32, tag="drec", bufs=2,
                               name=f"drec_{p}_{ks}")
                rview = drec.rearrange("a b -> (a b)").rearrange(
                    "(p f) -> p f", p=CHUNK)
                nc.sync.dma_start(out=rview, in_=rsb)
                bcf = wp.tile([CHUNK, SST], F32, tag="bcf", bufs=2,
                              name=f"bcf_{p}_{ks}")
                nc.sync.dma_start(
                    out=bcf[0:D, :],
                    in_=drec[0:1, 0:SST].broadcast_to((D, SST)))
                nc.sync.dma_start(
                    out=bcf[D:CHUNK, :],
                    in_=drec[0:1, SST:2 * SST].broadcast_to((D, SST)))
                pending_mul.append((p, ks, bcf))

            # =========== schedule ===========
            # pre-loop: full projection of strip 0
            for pj in (1, 0):
                for m in range(2):
                    proj_qk_group(0, pj, m, x8_bufs[0])
            for u in range(4):
                proj_v_group(0, u, xb_bufs[0])

            av_cur = [None]
            for ks in range(nks):
                if ks == 0:
                    load_wo()
                # prefetch x strip ks+2 (its buffers were freed by proj(ks))
                if ks + 2 < nks:
                    x8_bufs[ks + 2] = load_x8(ks + 2, [2, 2], [nc.scalar])
                    xb_bufs[ks + 2] = load_xb(ks + 2, [4, 4], [nc.scalar])

                fillers = []
                if ks + 1 < nks:                  # projection of next strip
                    for pj in (1, 0):
                        for m in range(2):
                            fillers.append(
                                (proj_qk_group, (ks + 1, pj, m, x8_bufs[ks + 1])))
                    for u in range(4):
                        fillers.append(
                            (proj_v_group, (ks + 1, u, xb_bufs[ks + 1])))
                if ks == nks - 1:                 # deferred out-proj
                    for jt in range(4 * (nks - 1)):
                        fillers.append((outproj_group, (jt,)))

                nj = 4 * ks + 4
                total_js = 2 * nj
                gi = 0
                cnt = 0
                paired = ks >= 1
                nu = nj // 2
                # Flat cross-pair schedule: pair 1's scores overlap pair 0's
                # AV tail + normalize, hiding the pair-boundary latency.
                steps = [(0, jj) for jj in range(nj)] + \
                        [(1, jj) for jj in range(nj)]
                due = {}
                last_av = {}
                for pi in (0, 1):
                    base = pi * nj
                    if paired:
                        for u in range(nu):
                            idx = base + 2 * u + 3
                            due.setdefault(idx, []).append(("pair", pi, u))
                            last_av[pi] = idx
                    else:
                        for jj in range(nj):
                            idx = base + jj + 2
                            due.setdefault(idx, []).append(("single", pi, jj))
                            last_av[pi] = idx
                av_tiles = {}

                def run_av(act, idx):
                    kind, pi, x = act
                    if pi not in av_tiles:
                        av_tiles[pi] = ps.tile([CHUNK, 2 * SST], F32,
                                               tag="av", bufs=1,
                                               name=f"av_{pi}_{ks}")
                    av_cur[0] = av_tiles[pi]
                    if kind == "pair":
                        av_pair(pi, ks, x, nu, epend.pop((pi, x)))
                    else:
                        av_step(pi, ks, x, nj)
                    if idx == last_av[pi] and pi == 0:
                        normalize(0, ks)

                res = 4 if ks == nks - 1 else 2
                cap = max(0, len(fillers) - res)
                for idx, (pi, jj) in enumerate(steps):
                    scores_step(pi, ks, jj)
                    for act in due.pop(idx, []):
                        run_av(act, idx)
                    if jj == 3:
                        flush_mul()   # deferred aoT mul from earlier pair
                    cnt += 1
                    want = min(cap,
                               ((cnt + 2) * len(fillers)) // total_js)
                    while gi < want:
                        fn, args = fillers[gi]
                        fn(*args)
                        gi += 1
                for idx in sorted(due):
                    for act in due[idx]:
                        run_av(act, idx)
                av_cur[0] = av_tiles[1]
                normalize(1, ks, tail=(ks == nks - 1))
                # keep the tensor queue fed while av drains
                while gi < len(fillers):
                    fn, args = fillers[gi]
                    fn(*args)
                    gi += 1

            # tail: out-proj of the last strip
            for jt in range(4 * (nks - 1), 4 * nks):
                outproj_group(jt, tail=True)

    if fix_waits:
        split_excess_waits(nc)
    return nc


def make_in_maps(x, wq, bq, wk, bk, wv, bv, wo, bo, t_len=T):
    """Build the 8 per-core input dicts from full inputs."""
    in_maps = []
    NCP = C // 256

    for core in range(NCORES):
        b, hg = core // 4, core % 4
        sl = slice(DPC * hg, DPC * (hg + 1))
        wqk = np.concatenate(
            [wq[sl].T, wk[sl].T], axis=1).astype(np.float32) * WSC
        w8 = np.ascontiguousarray(
            wqk.reshape(NCP, 2, CHUNK, -1).transpose(2, 0, 1, 3)).astype(F8_NP)
        bqs = (bq[sl] / 8.0).astype(np.float32)
        bqkm = np.stack([bqs[0:CHUNK], bqs[CHUNK:2 * CHUNK],
                         bk[sl][0:CHUNK], bk[sl][CHUNK:2 * CHUNK]], axis=1)
        xTb = np.ascontiguousarray(x[b, :t_len].T)
        x8r = np.ascontiguousarray(
            xTb.reshape(NCP, 2, CHUNK, t_len).transpose(2, 0, 1, 3)
        ).astype(F8_NP)
        in_maps.append({
            "x8": x8r,
            "w8": w8,
            "xT": xTb.astype(BF16_NP),
            "wvT": np.ascontiguousarray(wv[sl].T * WSC).astype(BF16_NP),
            "woT": np.ascontiguousarray(wo[:, sl].T).astype(BF16_NP),
            "bqk": np.ascontiguousarray(bqkm, dtype=np.float32),
            "bv_row": np.ascontiguousarray(
                (bv[sl] * WSC)[None, :], dtype=np.float32),
        })
    return in_maps


def gather_output(results, bo, t_len=T):
    ys = [np.asarray(results[i]["y"], dtype=np.float32) for i in range(NCORES)]
    out = np.stack([ys[0] + ys[1] + ys[2] + ys[3],
                    ys[4] + ys[5] + ys[6] + ys[7]])
    out += np.asarray(bo, np.float32)[None, None, :]
    return out


_NC_CACHE = {}


def _get_nc(t_len=T):
    if t_len not in _NC_CACHE:
        _NC_CACHE[t_len] = build_nc(t_len)
    return _NC_CACHE[t_len]


def kernel(x, wq, bq, wk, bk, wv, bv, wo, bo, mask=None, **_unused):
    """Full-input entry point: shard, run on 8 NeuronCores, gather."""
    from concourse.bass_utils import run_bass_kernel_spmd

    x = np.asarray(x, dtype=np.float32)
    in_maps = make_in_maps(x, np.asarray(wq, np.float32), np.asarray(bq, np.float32),
                           np.asarray(wk, np.float32), np.asarray(bk, np.float32),
                           np.asarray(wv, np.float32), np.asarray(bv, np.float32),
                           np.asarray(wo, np.float32), np.asarray(bo, np.float32))
    nc = _get_nc(T)
    res = run_bass_kernel_spmd(nc, in_maps, list(range(NCORES)))
    return gather_output(res.results, bo)


# revision 31
# speedup vs baseline: 1.0691x; 1.0140x over previous
"""Trainium2 Bass kernel: causal multi-head attention block (B=2, T=2048, C=1024, H=16).

Sharding: 8 cores = 2 (batch) x 4 (head groups of 4 heads).  Each core computes
q/k/v projections for its 4 heads, causal attention, and a partial out-proj
(rows of wo for its head slice).  Host sums the 4 partials per batch element.

v6: v3's software-pipelined schedule plus two PE packs, with precision kept
where fp8 noise does not average out:
  - q/k projections in fp8e4 with perf_mode=DoubleRow (contraction pairs
    packed 2-per-PE-cell: 4 matmuls per 1024-deep projection instead of 8).
    Weights scaled x64 on the host (fp8e4 subnormal cutoff 2^-6); the 1/64
    rides the existing bias-add.  Softmax is insensitive to the ~4% fp8
    element noise on scores (row-common factors cancel; diffuse rows
    average), unlike the v/out paths where fp8 noise lands directly on the
    output (measured ~5e-2 rel err) — so v-proj and out-proj stay bf16.
  - scores via PE row tiling: head A in PE rows 0:64, head B in rows 64:128
    (qT/kT stored as stacked pair tiles); the two score matmuls issue
    back-to-back and run concurrently in the array (~2x).  Head B keeps its
    own PSUM bank (same-bank packing + concurrency hangs the NEFF).
  - one strided exp activation per j-step ([A|B] banks in a single 3D AP);
    Act engine runs exp only, copies live on DVE.

Per-core layouts:
  x8      [128, 4, 2, 2048]  x[b].T partition-major c-pairs      (fp8e4)
  w8      [128, 4, 2, 512]   [64*wq_s.T | 64*wk_s.T] pairs       (fp8e4)
  xT      [1024, 2048]       x[b].T                              (bf16)
  wvT     [1024, 256]        wv_s.T                              (bf16)
  woT     [256, 1024]        wo[:, head_slice].T                 (bf16)
  bqk     [128, 4]  cols: bq/8 (pair0,pair1), bk (pair0,pair1)   (f32)
  bv_row  [1, 256]  bv                                           (f32)
  y       [2048, 1024]  partial output (pre-sum, pre-bo)         (bf16)
"""

import os
import sys

import numpy as np
import ml_dtypes

F8_NP = ml_dtypes.float8_e4m3   # TRN fp8e4: max 240, inf at 256
BF16_NP = ml_dtypes.bfloat16

for _p in ("/opt/trn_rl_repo", "/root/.axon_site/_ro/trn_rl_repo"):
    if os.path.isdir(_p) and _p not in sys.path:
        sys.path.append(_p)

import concourse.bass as bass  # noqa: E402
import concourse.mybir as mybir  # noqa: E402
import concourse.tile as tile  # noqa: E402

F32 = mybir.dt.float32
BF16 = mybir.dt.bfloat16
FP8 = mybir.dt.float8e4
DR = mybir.MatmulPerfMode.DoubleRow
MUL = mybir.AluOpType.mult
ADD = mybir.AluOpType.add

B, T, C, H = 2, 2048, 1024, 16
D = C // H          # 64
HPC = 4             # heads per core
DPC = HPC * D       # 256 head-dims per core
NCORES = 8

CHUNK = 128         # s-chunk / contraction granularity
SST = 512           # attention t-strip == one PSUM bank of f32
VW = 2 * D          # 128 per head: [ones x64 | v x64]
WSC = 64.0          # host-side fp8 weight scale

_CTRL_TYPES = (mybir.InstDrain, mybir.InstNoOp, mybir.InstEventSemaphore)


def split_excess_waits(nc, lim=1):
    """Walrus accepts at most one sync-wait per instruction; move extras onto
    same-engine NoOps inserted just before the owner."""
    k = 0
    for fn in nc.m.functions:
        for blk in fn.blocks:
            out = []
            changed = False
            for inst in blk.instructions:
                si = inst.sync_info
                if si is not None and si.on_wait and len(si.on_wait) > lim:
                    waits = list(si.on_wait)
                    extra, keep = waits[:-lim], waits[-lim:]
                    for w in extra:
                        nop = mybir.InstNoOp(name=f"waitfix_{k}", ins=[], outs=[])
                        k += 1
                        nop.engine = inst.engine
                        nop.sync_info = mybir.SyncInfo(on_wait=[w], on_update=[])
                        out.append(nop)
                    si.on_wait = keep
                    changed = True
                out.append(inst)
            if changed:
                blk.instructions = out
    return k


def build_nc(t_len=T, fix_waits=True):
    """Build the per-core SPMD Bass program (same program on all 8 cores)."""
    assert t_len % SST == 0
    nks = t_len // SST                # 4 strips
    n_cchunk = C // CHUNK             # 8
    n_ttile = t_len // CHUNK          # 16
    NCP = C // 256                    # 4 contraction pairs

    nc = bass.Bass(target_bir_lowering=False)

    x8 = nc.dram_tensor("x8", [CHUNK, NCP, 2, t_len], FP8, kind="ExternalInput")
    w8 = nc.dram_tensor("w8", [CHUNK, NCP, 2, 3 * DPC], FP8, kind="ExternalInput")
    xT = nc.dram_tensor("xT", [C, t_len], BF16, kind="ExternalInput")
    wvT = nc.dram_tensor("wvT", [C, DPC], BF16, kind="ExternalInput")
    woT = nc.dram_tensor("woT", [DPC, C], BF16, kind="ExternalInput")
    bqk = nc.dram_tensor("bqk", [CHUNK, 4], F32, kind="ExternalInput")
    bv_row = nc.dram_tensor("bv_row", [1, DPC], F32, kind="ExternalInput")
    y = nc.dram_tensor("y", [t_len, C], BF16, kind="ExternalOutput")

    Exp = mybir.ActivationFunctionType.Exp
    XW = 2 * NCP * SST               # per-strip fp8 x tile cols
    WW = 2 * NCP * 3 * DPC

    with tile.TileContext(nc) as tc:
        with tc.tile_pool(name="persist", bufs=1) as pp, \
             tc.tile_pool(name="work", bufs=1) as wp, \
             tc.tile_pool(name="dr", bufs=1, space="DRAM") as dr, \
             tc.tile_pool(name="ps", bufs=1, space="PSUM") as ps:
            # scalar-engine warmup: trigger the exp table load early
            warm = pp.tile([1, 8], F32, tag="warm", name="warm")
            nc.gpsimd.memset(warm, 1.0)
            nc.scalar.activation(warm, warm, mybir.ActivationFunctionType.Ln)
            nc.scalar.activation(warm, warm, Exp)

            # ---- input DMAs (spread across idle engine queues) ----
            bqk_sb = pp.tile([CHUNK, 4], F32, tag="bqk", name="bqk_sb")
            bv_bc = pp.tile([CHUNK, DPC], F32, tag="bv_bc", name="bv_bc")

            def load_x8(strip, parts, engs):
                """Load fp8 x strip (all NCP c-pairs) into one wide tile."""
                xts = wp.tile([CHUNK, XW], FP8, tag="x8s", bufs=2,
                              name=f"x8s_{strip}")
                c0 = 0
                for i, n in enumerate(parts):
                    engs[i % len(engs)].dma_start(
                        out=xts[:, c0 * 2 * SST:(c0 + n) * 2 * SST].rearrange(
                            "p (c i t) -> p c i t", i=2, t=SST),
                        in_=x8[:, c0:c0 + n, :,
                               strip * SST:(strip + 1) * SST])
                    c0 += n
                return xts

            def load_xb(strip, parts, engs):
                """Load bf16 x strip row-chunks (for the v projection)."""
                xbs = wp.tile([CHUNK, n_cchunk * SST], BF16, tag="xbs", bufs=2,
                              name=f"xbs_{strip}")
                c0 = 0
                for i, n in enumerate(parts):
                    engs[i % len(engs)].dma_start(
                        out=xbs[:, c0 * SST:(c0 + n) * SST].rearrange(
                            "p (c t) -> p c t", t=SST),
                        in_=xT[c0 * CHUNK:(c0 + n) * CHUNK,
                               strip * SST:(strip + 1) * SST].rearrange(
                            "(c p) t -> p c t", p=CHUNK))
                    c0 += n
                return xbs

            nc.gpsimd.dma_start(out=bqk_sb, in_=bqk[:, :])
            nc.gpsimd.dma_start(out=bv_bc, in_=bv_row[0:1, :].broadcast_to((CHUNK, DPC)))
            wts = pp.tile([CHUNK, WW], FP8, tag="wts", name="wts")
            wvb = pp.tile([CHUNK, n_cchunk * DPC], BF16, tag="wvb", name="wvb")
            x8s0 = wp.tile([CHUNK, XW], FP8, tag="x8s", bufs=2, name="x8s_0")
            rr = [nc.gpsimd, nc.sync, nc.scalar]
            qi = 0
            for c in range(NCP):
                rr[qi % 3].dma_start(
                    out=wts[:, c * 2 * 3 * DPC:(c + 1) * 2 * 3 * DPC],
                    in_=w8[:, c, :, :])
                qi += 1
                rr[qi % 3].dma_start(
                    out=x8s0[:, c * 2 * SST:(c + 1) * 2 * SST].rearrange(
                        "p (i t) -> p i t", i=2),
                    in_=x8[:, c, :, 0:SST])
                qi += 1
            nc.gpsimd.dma_start(
                out=wvb.rearrange("p (c w) -> p c w", w=DPC),
                in_=wvT[:, :].rearrange("(c p) w -> p c w", p=CHUNK))
            xb0 = load_xb(0, [4, 4], [nc.sync, nc.scalar])
            wtv = wts.rearrange("p (c i w) -> p c i w", i=2, w=3 * DPC)
            wv_sb = [wvb[:, c * DPC:(c + 1) * DPC] for c in range(n_cchunk)]

            def xtv(xts):
                return xts.rearrange("p (c i t) -> p c i t", i=2, t=SST)

            x8_bufs = {0: x8s0}
            xb_bufs = {0: xb0}
            if nks > 1:
                x8_bufs[1] = load_x8(1, [2, 2], [nc.sync, nc.scalar])

            wot = pp.tile([CHUNK, 2 * C], BF16, tag="wot", name="wot")
            wo_sb = [wot[:, 0:C], wot[:, C:2 * C]]

            def load_wo():
                nc.gpsimd.dma_start(
                    out=wot.rearrange("p (c w) -> p c w", w=C),
                    in_=woT[:, :].rearrange("(c p) w -> p c w", p=CHUNK))

            # HAM warm-up: keep the PE busy on throwaway matmuls while the
            # input DMAs land, so real work starts at 2.4 GHz
            dum = pp.tile([CHUNK, SST], BF16, tag="dum", name="dum")
            nc.vector.memset(dum, 0.0)
            for i in range(10):
                pd = ps.tile([CHUNK, SST], F32, tag="fill", bufs=2,
                             name=f"pd_{i}")
                nc.tensor.matmul(pd, dum[:, 0:CHUNK], dum, start=True, stop=True)

            # ---- persistent activations ----
            # qT2/kT2[m]: heads 2m / 2m+1 stacked on partitions 0:64 / 64:128
            qT2 = [pp.tile([CHUNK, t_len], BF16, tag=f"qT{m}", name=f"qT{m}")
                   for m in range(2)]
            kT2 = [pp.tile([CHUNK, t_len], BF16, tag=f"kT{m}", name=f"kT{m}")
                   for m in range(2)]
            # strip-0 AV runs bf16 (its short rows are fp8-noise-sensitive);
            # strips >=1 run fp8 DoubleRow over s-chunk PAIRS.  Both vaug
            # forms hold [ones(=64) | 64*v] — the 64 cancels in the softmax
            # ratio (wvT/bv are scaled x64 on the host).
            vaug = [pp.tile([CHUNK, HPC * VW], BF16, tag=f"v{j}", name=f"v{j}")
                    for j in range(4)]
            for j in range(4):
                eng = nc.vector if j % 2 == 0 else nc.gpsimd
                eng.memset(vaug[j], WSC)   # ones half survives; rest overwritten
            vaug2 = [pp.tile([CHUNK, HPC * 2 * VW], FP8, tag=f"w2{u}",
                             name=f"v2{u}")
                     for u in range(n_ttile // 2)]
            for u in range(n_ttile // 2):
                eng = nc.vector if u % 2 == 0 else nc.gpsimd
                eng.memset(vaug2[u], WSC)
            aoT = [pp.tile([CHUNK, t_len], BF16, tag=f"aoT{p}", name=f"aoT{p}")
                   for p in range(2)]
            # 0/1 lower-triangle for diagonal-tile masking (DVE multiply —
            # keeps the gpsimd queue out of the per-step critical path)
            tri = pp.tile([CHUNK, CHUNK], BF16, tag="tri", name="tri")
            nc.gpsimd.memset(tri, 1.0)
            nc.gpsimd.affine_select(
                out=tri, in_=tri, pattern=[[1, CHUNK]], channel_multiplier=-1,
                base=0, compare_op=mybir.AluOpType.is_ge, fill=0.0)

            # =========== emit helpers ===========
            def proj_qk_group(strip, pj, m, xts):
                """q or k (pj=0/1) for head-pair m over one t-strip: fp8 DR."""
                xt = xtv(xts)
                pq = ps.tile([CHUNK, SST], F32, tag="fill", bufs=2,
                             name=f"pq{pj}{m}_{strip}")
                base = pj * DPC + m * CHUNK
                for cp in range(NCP):
                    nc.tensor.matmul(
                        pq,
                        wtv[:, cp, :, base:base + CHUNK],
                        xt[:, cp, :, :],
                        start=(cp == 0), stop=(cp == NCP - 1),
                        perf_mode=DR)
                dst = kT2[m] if pj else qT2[m]
                # q additionally carries the softmax 1/sqrt(64)=1/8
                sc = 1.0 / (WSC * 8.0) if pj == 0 else 1.0 / WSC
                nc.vector.tensor_scalar(
                    dst[:, strip * SST:(strip + 1) * SST], pq,
                    sc, bqk_sb[:, 2 * pj + m:2 * pj + m + 1], MUL, ADD)

            def proj_v_group(strip, u, xbs):
                """64*v for t-chunk 4*strip+u -> vaug2 (+ bf16 vaug for strip
                0, whose short attention rows are fp8-noise-sensitive; the
                other strips' vaug2 is e4m3 anyway, so their v matmuls run
                fp8 DoubleRow from x8/w8)."""
                jt = 4 * strip + u
                pv = ps.tile([CHUNK, SST], F32, tag="fill", bufs=2,
                             name=f"pv_{jt}")
                if strip == 0:
                    for c in range(n_cchunk):
                        nc.tensor.matmul(
                            pv[:, 0:DPC],
                            xbs[:, c * SST + u * CHUNK:
                                c * SST + (u + 1) * CHUNK],
                            wv_sb[c],
                            start=(c == 0), stop=(c == n_cchunk - 1))
                else:
                    xt = xtv(xbs)
                    for cp in range(NCP):
                        nc.tensor.matmul(
                            pv[:, 0:DPC],
                            xt[:, cp, :, u * CHUNK:(u + 1) * CHUNK],
                            wtv[:, cp, :, 2 * DPC:3 * DPC],
                            start=(cp == 0), stop=(cp == NCP - 1),
                            perf_mode=DR)
                u2, i2 = jt // 2, jt % 2
                nc.vector.tensor_add(
                    vaug2[u2].rearrange("p (h i e) -> p h i e", i=2, e=VW)
                    [:, :, i2, D:2 * D],
                    pv[:, 0:DPC].rearrange("p (h d) -> p h d", d=D),
                    bv_bc.rearrange("p (h d) -> p h d", d=D))
                if jt < 4:   # strip-0 also needs the bf16 copy
                    nc.vector.tensor_add(
                        vaug[jt].rearrange("p (h e) -> p h e", e=VW)
                        [:, :, D:2 * D],
                        pv[:, 0:DPC].rearrange("p (h d) -> p h d", d=D),
                        bv_bc.rearrange("p (h d) -> p h d", d=D))

            def outproj_group(jt, tail=False):
                for js in range(2):
                    py = ps.tile([CHUNK, SST], F32, tag="fill", bufs=2,
                                 name=f"py_{jt}_{js}")
                    for p in range(2):
                        nc.tensor.matmul(
                            py,
                            aoT[p][:, jt * CHUNK:(jt + 1) * CHUNK],
                            wo_sb[p][:, js * SST:(js + 1) * SST],
                            start=(p == 0), stop=(p == 1))
                    ysb = wp.tile([CHUNK, SST], BF16, tag="ysb", bufs=3,
                                  name=f"ysb_{jt}_{js}")
                    if tail and (2 * jt + js) % 2 == 0:
                        # the scalar engine is idle in the tail; splitting the
                        # PSUM evictions halves the vector-bound epilogue
                        nc.scalar.activation(ysb, py,
                                             mybir.ActivationFunctionType.Copy)
                    else:
                        nc.vector.tensor_copy(ysb, py)
                    if tail:
                        eng = [nc.sync, nc.scalar][(2 * jt + js) % 2]
                    else:
                        eng = [nc.gpsimd, nc.sync][(2 * jt + js) % 2]
                    eng.dma_start(
                        out=y[jt * CHUNK:(jt + 1) * CHUNK,
                              js * SST:(js + 1) * SST],
                        in_=ysb)

            pend = {}
            epend = {}

            def scores_step(p, ks, j):
                """scores -> exp -> (mask) for one s-chunk j of strip ks.

                Scores for heads 2p / 2p+1 run concurrently via PE row tiling
                (kT2/qT2 partition halves); head B in its own PSUM bank.
                One strided exp covers both banks.  For strips >= 1 the exp
                writes fp8 into the shared chunk-PAIR tile ePair (layout
                [i(chunk) | head | t]) consumed later by a DoubleRow AV; the
                pair shares the even chunk's t-window, with the odd chunk's
                leading CHUNK columns masked off.  Strip 0 keeps the bf16
                per-chunk path.
                """
                paired = ks >= 1
                jb = (j - (j % 2)) if paired else j
                off = max(0, CHUNK * jb - SST * ks)
                L = SST - off
                t0 = SST * ks + off
                jc = j * CHUNK
                diag = CHUNK * jb >= SST * ks
                sAB = ps.tile([CHUNK, 2 * SST], F32, tag="sAB", bufs=2,
                              name=f"s_{p}_{ks}_{j}")
                nc.tensor.matmul(
                    sAB[:, 0:L],
                    kT2[p][0:D, jc:jc + CHUNK],
                    qT2[p][0:D, t0:t0 + L],
                    start=True, stop=True, skip_group_check=True)
                nc.tensor.matmul(
                    sAB[:, SST:SST + L],
                    kT2[p][D:CHUNK, jc:jc + CHUNK],
                    qT2[p][D:CHUNK, t0:t0 + L],
                    start=True, stop=True, skip_group_check=True)
                s_in = sAB.rearrange("p (c t) -> p c t", t=SST)[:, :, 0:L]
                if not paired:
                    eAB = wp.tile([CHUNK, 2 * SST], BF16, tag="eAB", bufs=3,
                                  name=f"e_{p}_{ks}_{j}")
                    if L == SST:
                        nc.scalar.activation(eAB, sAB, Exp)
                    else:
                        nc.scalar.activation(
                            eAB[:, 0:2 * L].rearrange(
                                "p (c t) -> p c t", c=2), s_in, Exp)
                    if diag:
                        for base in (0, L):
                            nc.vector.tensor_mul(
                                eAB[:, base:base + CHUNK],
                                eAB[:, base:base + CHUNK], tri)
                    pend[(p, j)] = (eAB, off, L)
                    return
                i = j % 2
                if i == 0:
                    eP = wp.tile([CHUNK, 4 * SST], FP8, tag="eP", bufs=3,
                                 name=f"eP_{p}_{ks}_{j}")
                    epend[(p, j // 2)] = (eP, off, L)
                else:
                    eP = epend[(p, j // 2)][0]
                ePv = eP.rearrange("p (i c t) -> p i c t", i=2, c=2)
                nc.scalar.activation(ePv[:, i, :, 0:L], s_in, Exp)
                if diag:
                    if i == 0:
                        for c in range(2):
                            nc.vector.tensor_mul(
                                ePv[:, 0, c, 0:CHUNK],
                                ePv[:, 0, c, 0:CHUNK], tri)
                    else:
                        for c in range(2):
                            nc.vector.memset(ePv[:, 1, c, 0:CHUNK], 0.0)
                            nc.vector.tensor_mul(
                                ePv[:, 1, c, CHUNK:2 * CHUNK],
                                ePv[:, 1, c, CHUNK:2 * CHUNK], tri)

            def av_step(p, ks, j, nj):
                hA, hB = 2 * p, 2 * p + 1
                eAB, off, L = pend.pop((p, j))
                av = av_cur[0]
                nc.tensor.matmul(
                    av[:, off:SST],
                    vaug[j][:, hA * VW:(hA + 1) * VW],
                    eAB[:, 0:L],
                    start=(j == 0), stop=(j == nj - 1), skip_group_check=True)
                nc.tensor.matmul(
                    av[:, SST + off:2 * SST],
                    vaug[j][:, hB * VW:(hB + 1) * VW],
                    eAB[:, L:2 * L],
                    start=(j == 0), stop=(j == nj - 1), skip_group_check=True)

            def av_pair(p, ks, u, nu, pe):
                """fp8 DoubleRow AV over s-chunk pair u (strips >= 1)."""
                eP, off, L = pe
                av = av_cur[0]
                ePv = eP.rearrange("p (i c t) -> p i c t", i=2, c=2)
                for c in range(2):
                    nc.tensor.matmul(
                        av[:, c * SST + off:(c + 1) * SST],
                        vaug2[u][:, (2 * p + c) * 2 * VW:
                                 (2 * p + c + 1) * 2 * VW]
                        .rearrange("p (i e) -> p i e", i=2),
                        ePv[:, :, c, 0:L],
                        start=(u == 0), stop=(u == nu - 1),
                        perf_mode=DR, skip_group_check=True)

            pending_mul = []

            def flush_mul():
                while pending_mul:
                    p, ks, bcf = pending_mul.pop(0)
                    strip = slice(ks * SST, (ks + 1) * SST)
                    half = SST // 2
                    lo = ks * SST
                    nc.gpsimd.tensor_mul(aoT[p][:, lo:lo + half],
                                         aoT[p][:, lo:lo + half],
                                         bcf[:, 0:half])
                    nc.gpsimd.tensor_mul(aoT[p][:, lo + half:lo + SST],
                                         aoT[p][:, lo + half:lo + SST],
                                         bcf[:, half:SST])

            def normalize(p, ks, tail=False):
                """softmax-normalize strip ks of pair p into aoT[p].

                av rows 0:64 hold the PE-replicated denominators, rows 64:128
                the unnormalized outputs.  Reciprocals run directly on the
                PSUM denominator rows (the DVE 32-lane shuffle crossbar
                handles the 64-partition shift), no DRAM bounce needed.  The
                final aoT multiply is deferred (flush_mul) so it never blocks
                the next pair's masks on the gpsimd queue — except in the
                tail, where it runs immediately."""
                av = av_cur[0]
                strip = slice(ks * SST, (ks + 1) * SST)
                half = SST // 2
                lo = ks * SST
                if tail:
                    # low-latency path: 1/d = exp(-log d) on the (idle) scalar
                    # (Ln of the A-half reads the PSUM denominator directly;
                    # the B-half needs the DVE crossbar for the 64-partition
                    # shift, so it goes through a copy)
                    rec = wp.tile([CHUNK, SST], F32, tag="rec", bufs=2,
                                  name=f"rec_{p}_{ks}")
                    nc.scalar.activation(rec[0:D, :], av[0:D, 0:SST],
                                         mybir.ActivationFunctionType.Ln)
                    nc.vector.tensor_copy(rec[D:CHUNK, :], av[0:D, SST:2 * SST])
                    nc.vector.tensor_copy(aoT[p][0:D, strip],
                                          av[D:CHUNK, 0:SST])
                    nc.scalar.activation(rec[D:CHUNK, :], rec[D:CHUNK, :],
                                         mybir.ActivationFunctionType.Ln)
                    nc.vector.tensor_copy(aoT[p][D:CHUNK, strip],
                                          av[D:CHUNK, SST:2 * SST])
                    nc.scalar.activation(rec, rec, Exp, scale=-1.0)
                    nc.gpsimd.tensor_mul(aoT[p][:, lo:lo + half],
                                         aoT[p][:, lo:lo + half],
                                         rec[:, 0:half])
                    nc.vector.tensor_mul(aoT[p][:, lo + half:lo + SST],
                                         aoT[p][:, lo + half:lo + SST],
                                         rec[:, half:SST])
                    return
                den = wp.tile([1, 2 * SST], F32, tag="den", bufs=2,
                              name=f"den_{p}_{ks}")
                # evicts; frees the av banks.  B-side evict on the scalar
                # engine (idle at pair boundaries).
                nc.vector.tensor_copy(aoT[p][0:D, strip],
                                      av[D:CHUNK, 0:SST])
                nc.scalar.activation(aoT[p][D:CHUNK, strip],
                                     av[D:CHUNK, SST:2 * SST],
                                     mybir.ActivationFunctionType.Copy)
                nc.vector.tensor_copy(den[0:1, 0:SST], av[0:1, 0:SST])
                nc.vector.tensor_copy(den[0:1, SST:2 * SST],
                                      av[0:1, SST:2 * SST])
                # reciprocal via a [128, 8] reshape (wide on the DVE lanes),
                # then partition-broadcast — both through small DRAM bounces
                # (SBUF APs cannot reshape across partitions / stride-0 bcast).
                # Latency is hidden: the aoT multiply is deferred into the
                # next pair's j-loop (flush_mul).
                dden = dr.tile([1, 2 * SST], F32, tag="dden", bufs=2,
                               name=f"dden_{p}_{ks}")
                nc.sync.dma_start(out=dden, in_=den)
                nf = 2 * SST // CHUNK
                dsb = wp.tile([CHUNK, nf], F32, tag="dsb", bufs=2,
                              name=f"dsb_{p}_{ks}")
                dview = dden.rearrange("a b -> (a b)").rearrange(
                    "(p f) -> p f", p=CHUNK)
                nc.sync.dma_start(out=dsb, in_=dview)
                rsb = wp.tile([CHUNK, nf], F32, tag="rsb", bufs=2,
                              name=f"rsb_{p}_{ks}")
                nc.vector.reciprocal(rsb, dsb)
                drec = dr.tile([1, 2 * SST], F32, tag="drec", bufs=2,
                               name=f"drec_{p}_{ks}")
                rview = drec.rearrange("a b -> (a b)").rearrange(
                    "(p f) -> p f", p=CHUNK)
                nc.sync.dma_start(out=rview, in_=rsb)
                bcf = wp.tile([CHUNK, SST], F32, tag="bcf", bufs=2,
                              name=f"bcf_{p}_{ks}")
                nc.sync.dma_start(
                    out=bcf[0:D, :],
                    in_=drec[0:1, 0:SST].broadcast_to((D, SST)))
                nc.sync.dma_start(
                    out=bcf[D:CHUNK, :],
                    in_=drec[0:1, SST:2 * SST].broadcast_to((D, SST)))
                pending_mul.append((p, ks, bcf))

            # =========== schedule ===========
            # pre-loop: full projection of strip 0
            for pj in (1, 0):
                for m in range(2):
                    proj_qk_group(0, pj, m, x8_bufs[0])
            for u in range(4):
                proj_v_group(0, u, xb_bufs[0])

            av_cur = [None]
            for ks in range(nks):
                if ks == 0:
                    load_wo()
                # prefetch x strip ks+2 (its buffers were freed by proj(ks))
                if ks + 2 < nks:
                    x8_bufs[ks + 2] = load_x8(ks + 2, [2, 2], [nc.scalar])

                fillers = []
                if ks + 1 < nks:                  # projection of next strip
                    for pj in (1, 0):
                        for m in range(2):
                            fillers.append(
                                (proj_qk_group, (ks + 1, pj, m, x8_bufs[ks + 1])))
                    for u in range(4):
                        fillers.append(
                            (proj_v_group, (ks + 1, u, x8_bufs[ks + 1])))
                if ks == nks - 1:                 # deferred out-proj
                    for jt in range(4 * (nks - 1)):
                        fillers.append((outproj_group, (jt,)))

                nj = 4 * ks + 4
                total_js = 2 * nj
                gi = 0
                cnt = 0
                paired = ks >= 1
                nu = nj // 2
                # Flat cross-pair schedule: pair 1's scores overlap pair 0's
                # AV tail + normalize, hiding the pair-boundary latency.
                steps = [(0, jj) for jj in range(nj)] + \
                        [(1, jj) for jj in range(nj)]
                due = {}
                last_av = {}
                for pi in (0, 1):
                    base = pi * nj
                    if paired:
                        for u in range(nu):
                            idx = base + 2 * u + 3
                            due.setdefault(idx, []).append(("pair", pi, u))
                            last_av[pi] = idx
                    else:
                        for jj in range(nj):
                            idx = base + jj + 2
                            due.setdefault(idx, []).append(("single", pi, jj))
                            last_av[pi] = idx
                av_tiles = {}

                def run_av(act, idx):
                    kind, pi, x = act
                    if pi not in av_tiles:
                        av_tiles[pi] = ps.tile([CHUNK, 2 * SST], F32,
                                               tag="av", bufs=1,
                                               name=f"av_{pi}_{ks}")
                    av_cur[0] = av_tiles[pi]
                    if kind == "pair":
                        av_pair(pi, ks, x, nu, epend.pop((pi, x)))
                    else:
                        av_step(pi, ks, x, nj)
                    if idx == last_av[pi] and pi == 0:
                        normalize(0, ks)

                res = 4 if ks == nks - 1 else 2
                cap = max(0, len(fillers) - res)
                for idx, (pi, jj) in enumerate(steps):
                    scores_step(pi, ks, jj)
                    for act in due.pop(idx, []):
                        run_av(act, idx)
                    if jj == 3:
                        flush_mul()   # deferred aoT mul from earlier pair
                    cnt += 1
                    want = min(cap,
                               ((cnt + 2) * len(fillers)) // total_js)
                    while gi < want:
                        fn, args = fillers[gi]
                        fn(*args)
                        gi += 1
                for idx in sorted(due):
                    for act in due[idx]:
                        run_av(act, idx)
                av_cur[0] = av_tiles[1]
                normalize(1, ks, tail=(ks == nks - 1))
                # keep the tensor queue fed while av drains
                while gi < len(fillers):
                    fn, args = fillers[gi]
                    fn(*args)
                    gi += 1

            # tail: out-proj of the last strip
            for jt in range(4 * (nks - 1), 4 * nks):
                outproj_group(jt, tail=True)

    if fix_waits:
        split_excess_waits(nc)
    return nc


def make_in_maps(x, wq, bq, wk, bk, wv, bv, wo, bo, t_len=T):
    """Build the 8 per-core input dicts from full inputs."""
    in_maps = []
    NCP = C // 256

    for core in range(NCORES):
        b, hg = core // 4, core % 4
        sl = slice(DPC * hg, DPC * (hg + 1))
        wqk = np.concatenate(
            [wq[sl].T, wk[sl].T, wv[sl].T], axis=1).astype(np.float32) * WSC
        w8 = np.ascontiguousarray(
            wqk.reshape(NCP, 2, CHUNK, -1).transpose(2, 0, 1, 3)).astype(F8_NP)
        bqs = (bq[sl] / 8.0).astype(np.float32)
        bqkm = np.stack([bqs[0:CHUNK], bqs[CHUNK:2 * CHUNK],
                         bk[sl][0:CHUNK], bk[sl][CHUNK:2 * CHUNK]], axis=1)
        xTb = np.ascontiguousarray(x[b, :t_len].T)
        x8r = np.ascontiguousarray(
            xTb.reshape(NCP, 2, CHUNK, t_len).transpose(2, 0, 1, 3)
        ).astype(F8_NP)
        in_maps.append({
            "x8": x8r,
            "w8": w8,
            "xT": xTb.astype(BF16_NP),
            "wvT": np.ascontiguousarray(wv[sl].T * WSC).astype(BF16_NP),
            "woT": np.ascontiguousarray(wo[:, sl].T).astype(BF16_NP),
            "bqk": np.ascontiguousarray(bqkm, dtype=np.float32),
            "bv_row": np.ascontiguousarray(
                (bv[sl] * WSC)[None, :], dtype=np.float32),
        })
    return in_maps


def gather_output(results, bo, t_len=T):
    ys = [np.asarray(results[i]["y"], dtype=np.float32) for i in range(NCORES)]
    out = np.stack([ys[0] + ys[1] + ys[2] + ys[3],
                    ys[4] + ys[5] + ys[6] + ys[7]])
    out += np.asarray(bo, np.float32)[None, None, :]
    return out


_NC_CACHE = {}


def _get_nc(t_len=T):
    if t_len not in _NC_CACHE:
        _NC_CACHE[t_len] = build_nc(t_len)
    return _NC_CACHE[t_len]


def kernel(x, wq, bq, wk, bk, wv, bv, wo, bo, mask=None, **_unused):
    """Full-input entry point: shard, run on 8 NeuronCores, gather."""
    from concourse.bass_utils import run_bass_kernel_spmd

    x = np.asarray(x, dtype=np.float32)
    in_maps = make_in_maps(x, np.asarray(wq, np.float32), np.asarray(bq, np.float32),
                           np.asarray(wk, np.float32), np.asarray(bk, np.float32),
                           np.asarray(wv, np.float32), np.asarray(bv, np.float32),
                           np.asarray(wo, np.float32), np.asarray(bo, np.float32))
    nc = _get_nc(T)
    res = run_bass_kernel_spmd(nc, in_maps, list(range(NCORES)))
    return gather_output(res.results, bo)
